# revision 1
# baseline (speedup 1.0000x reference)
"""Trainium2 Bass kernel for a 3-layer GENConv-style GNN (DGCN) on 8 NeuronCores.

Strategy (graph-partition data parallel):
  - Nodes are split contiguously across 8 cores (6250 nodes/core); each core owns
    all edges whose *destination* lies in its node range.
  - Node state hv lives in SBUF as [H=128 partitions, nodes] per core.
  - Per layer: tiny AllReduce of BatchNorm statistics -> BN+ReLU in one ScalarE
    activation pass -> transpose hv1 tiles (scaled by beta via a beta*I identity)
    into a DRAM shard -> AllGather into a per-core full gather table [N, H] ->
    edge pass: indirect-DMA gather of hv1[src] rows with a second accumulating
    indirect gather adding the (per-layer) edge-embedding rows, ReLU and exp on
    ScalarE, m*e on VectorE, and segment sums via TensorE matmuls against
    host-precomputed 0/1 indicator tiles (edges pre-sorted by destination) ->
    agg = (sum m*e) / (beta * sum e) -> MLP matmul (+bias via a rank-1 matmul)
    + skip.
  - Softmax max-subtraction is skipped (z = beta*m is bounded, exp can't
    overflow; result is shift-invariant); empty segments guarded with +1e-30.
  - Final average pooling via a host-built 0/1 pooling matrix on TensorE; the
    per-core partial [16, H+1] (sums + counts) is AllReduced, and every core
    computes the output Linear.

The program is SPMD (one instruction stream for all 8 cores), so all
data-dependent tiling metadata (tiles per dst-block, indicator widths) is
uniformized across cores by taking per-(block,tile) maxima; the actual indices
and indicator contents are per-core input data.
"""

import sys

sys.path.insert(0, "/opt/trn_rl_repo")

import numpy as np
import ml_dtypes

import concourse.bass as bass
import concourse.bacc as bacc
import concourse.tile as tile
import concourse.mybir as mybir
from concourse import bass_utils
from concourse.masks import make_identity

f32 = mybir.dt.float32
bf16 = mybir.dt.bfloat16
i32 = mybir.dt.int32

N_CORES = 8
H = 128
P = 128
OUT_DIM = 14
NGRAPH = 16
LAYERS = 3
BN_EPS = 1e-5
GEN_EPS = 1e-7
DEN_TINY = 1e-30
DEBUG = False
ABLATE = set()  # timing ablations: "gather","elem","segmm","coll","node"


class Meta:
    pass


def _preprocess(inputs):
    """Host-side index preprocessing + per-core input shards."""
    m = Meta()
    src = np.asarray(inputs["edge_src"], np.int64)
    dst = np.asarray(inputs["edge_dst"], np.int64)
    combo = (np.asarray(inputs["edge_feat0"], np.int64) * 3
             + np.asarray(inputs["edge_feat1"], np.int64))
    nf0 = np.asarray(inputs["node_feat0"], np.int64)
    nf1 = np.asarray(inputs["node_feat1"], np.int64)
    gids = np.asarray(inputs["graph_ids"], np.int64)

    N = nf0.shape[0]
    V0 = np.asarray(inputs["W_node0"]).shape[0]
    V1 = np.asarray(inputs["W_node1"]).shape[0]
    NPC = N // N_CORES
    nblk = (NPC + P - 1) // P
    m.N, m.NPC, m.nblk = N, NPC, nblk
    m.V0, m.V1 = V0, V1

    order = np.argsort(dst, kind="stable")
    dsts, srcs, combos = dst[order], src[order], combo[order]

    SPLIT = 32768  # int16 gather-index limit; table accessed as two halves
    cnt = np.zeros((N_CORES, nblk), dtype=np.int64)
    rng = {}
    for c in range(N_CORES):
        for b in range(nblk):
            lo = c * NPC + b * P
            hi = min(c * NPC + min(NPC, (b + 1) * P), (c + 1) * NPC)
            i0 = np.searchsorted(dsts, lo, "left")
            i1 = np.searchsorted(dsts, hi, "left")
            rng[(c, b)] = (i0, i1)
            cnt[c, b] = i1 - i0
    m.dwin = [min(P, NPC - b * P) for b in range(nblk)]

    # per-(core, block) edges reordered: (src < SPLIT, by dst), then (src >= SPLIT, by dst)
    # pad each half to a cross-core-uniform number of 128-edge tiles.
    core_blk = {}
    nlo_t = np.ones(nblk, dtype=np.int64)
    nhi_t = np.ones(nblk, dtype=np.int64)
    for c in range(N_CORES):
        for b in range(nblk):
            i0, i1 = rng[(c, b)]
            s_b = srcs[i0:i1]
            d_b = dsts[i0:i1] - (c * NPC + b * P)
            c_b = combos[i0:i1]
            is_hi = s_b >= SPLIT
            lo_sel = np.argsort(d_b[~is_hi], kind="stable")
            hi_sel = np.argsort(d_b[is_hi], kind="stable")
            parts = ((s_b[~is_hi][lo_sel], d_b[~is_hi][lo_sel], c_b[~is_hi][lo_sel]),
                     (s_b[is_hi][hi_sel] - SPLIT, d_b[is_hi][hi_sel], c_b[is_hi][hi_sel]))
            core_blk[(c, b)] = parts
            nlo_t[b] = max(nlo_t[b], (len(parts[0][0]) + P - 1) // P)
            nhi_t[b] = max(nhi_t[b], (len(parts[1][0]) + P - 1) // P)
    m.nlo_t, m.nhi_t = nlo_t, nhi_t
    NT = nlo_t + nhi_t
    m.NT = NT

    # uniform covers per (b, run, t): cross-core dst spans; block-first/last
    # tiles full-width (PSUM zero/close discipline); middles quadrant-aligned.
    covers = []
    for b in range(nblk):
        spans = []
        for run, ntr in ((0, int(nlo_t[b])), (1, int(nhi_t[b]))):
            for t in range(ntr):
                lo_u, hi_u = 1 << 30, 0
                for c in range(N_CORES):
                    dl = core_blk[(c, b)][run][1]
                    ch = dl[t * P:(t + 1) * P]
                    if len(ch):
                        lo_u = min(lo_u, int(ch.min()))
                        hi_u = max(hi_u, int(ch.max() + 1))
                spans.append((lo_u, hi_u))
        cv = []
        ntot = int(NT[b])
        for t in range(ntot):
            lo_u, hi_u = spans[t]
            if t == 0 or t == ntot - 1:
                cv.append((0, m.dwin[b]))
            else:
                lo = min(lo_u, m.dwin[b] - 1)
                hi = max(min(hi_u, m.dwin[b]), lo + 1)
                a32 = (lo // 32) * 32
                a64 = (lo // 64) * 64
                if hi <= a32 + 32 and a32 in (0, 32, 64):
                    cv.append((a32, 32))
                elif hi <= a64 + 64:
                    cv.append((a64, 64))
                else:
                    cv.append((0, P))
        covers.append(cv)
    m.covers = covers
    m.T = int(NT.sum())
    woff, acc = [], 0
    for cv in covers:
        woff.append(acc)
        acc += sum(w for (_, w) in cv)
    m.woff = woff
    m.Wtot = acc
    m.Wmax = max(sum(w for (_, w) in cv) for cv in covers)

    # group consecutive blocks for batched gathers
    GT = 48
    groups, cur = [], None
    for b in range(nblk):
        if cur is None or cur["nt"] + int(NT[b]) > GT:
            cur = dict(blocks=[], nt=0, tlo=0, thi=0)
            groups.append(cur)
        cur["blocks"].append(b)
        cur["nt"] += int(NT[b])
        cur["tlo"] += int(nlo_t[b])
        cur["thi"] += int(nhi_t[b])
    for grp in groups:
        lt = ht = 0
        grp["lo_toff"], grp["hi_toff"] = {}, {}
        for b in grp["blocks"]:
            grp["lo_toff"][b] = lt; lt += int(nlo_t[b])
            grp["hi_toff"][b] = ht; ht += int(nhi_t[b])
    m.groups = groups
    m.GT = GT
    # flat offsets for idx / combo streams (units: 16-wrapped columns)
    ixoff, cxoff = [], []
    ax = cx = 0
    for grp in groups:
        ixoff.append(ax)
        cxoff.append(cx)
        ax += (grp["tlo"] + grp["thi"]) * 8
        cx += (grp["tlo"] + grp["thi"]) * 8
    m.ixoff, m.IXW = ixoff, ax
    m.cxoff, m.CXW = cxoff, cx

    def wrap16(seq):
        sq = seq.astype(np.int16).reshape(-1, 16).T        # [16, S]
        return np.tile(sq, (8, 1))                          # [128, S]

    # --- per-core data arrays ---
    per_core = []
    for c in range(N_CORES):
        gidx = np.zeros((P, m.IXW), np.int16)
        gcmb = np.zeros((P, m.CXW), np.int16)
        ind_blocks = []
        blk_streams = {}
        for b in range(nblk):
            (ls, ld, lc), (hs, hd, hc) = core_blk[(c, b)]
            streams = []
            for (s_, d_, c_), ntr in (((ls, ld, lc), int(nlo_t[b])),
                                      ((hs, hd, hc), int(nhi_t[b]))):
                npad = ntr * P - len(s_)
                streams.append((
                    np.concatenate([s_, np.zeros(npad, np.int64)]),
                    np.concatenate([d_, np.full(npad, -1, np.int64)]),
                    np.concatenate([c_, np.zeros(npad, np.int64)])))
            blk_streams[b] = streams
            # indicators (tile order: lo tiles then hi tiles)
            Wb = sum(w for (_, w) in covers[b])
            ind_b = np.zeros((P, Wb), np.float32)
            wo = 0
            tglob = 0
            for run in (0, 1):
                d_all = streams[run][1]
                ntr = int(nlo_t[b]) if run == 0 else int(nhi_t[b])
                for t in range(ntr):
                    lo, w = covers[b][tglob]
                    dt_ = d_all[t * P:(t + 1) * P]
                    cols = dt_ - lo
                    valid = (dt_ >= 0) & (cols >= 0) & (cols < w)
                    I = np.zeros((P, w), np.float32)
                    I[np.arange(P)[valid], cols[valid]] = 1.0
                    ind_b[:, wo : wo + w] = I
                    wo += w
                    tglob += 1
            ind_blocks.append(ind_b.astype(ml_dtypes.bfloat16).ravel())
        # idx / combo streams per group
        for gi, grp in enumerate(groups):
            lo_seq = np.concatenate(
                [blk_streams[b][0][0] for b in grp["blocks"]])
            hi_seq = np.concatenate(
                [blk_streams[b][1][0] for b in grp["blocks"]])
            cm_seq = np.concatenate(
                [blk_streams[b][0][2] for b in grp["blocks"]]
                + [blk_streams[b][1][2] for b in grp["blocks"]])
            o = m.ixoff[gi]
            gidx[:, o : o + grp["tlo"] * 8] = wrap16(lo_seq)
            gidx[:, o + grp["tlo"] * 8 : o + (grp["tlo"] + grp["thi"]) * 8] = \
                wrap16(hi_seq)
            o = m.cxoff[gi]
            gcmb[:, o : o + (grp["tlo"] + grp["thi"]) * 8] = wrap16(cm_seq)

        nlo, nhi = c * NPC, (c + 1) * NPC
        oh0 = np.zeros((V0, NPC), np.float32)
        oh0[nf0[nlo:nhi], np.arange(NPC)] = 1.0
        oh1 = np.zeros((V1, NPC), np.float32)
        oh1[nf1[nlo:nhi], np.arange(NPC)] = 1.0
        pmat = np.zeros((nblk * P, NGRAPH), np.float32)
        g_c = gids[nlo:nhi]
        for b in range(nblk):
            w = min(P, NPC - b * P)
            pmat[b * P : b * P + w, :][np.arange(w), g_c[b * P : b * P + w]] = 1.0
        per_core.append(dict(
            gidx=gidx, gcmb=gcmb,
            ind=np.concatenate(ind_blocks),
            oh0=oh0, oh1=oh1, pmat=pmat,
        ))

    # replicated parameter tensors
    ee0 = np.asarray(inputs["edge_emb0"], np.float32)   # [L, 6, H]
    ee1 = np.asarray(inputs["edge_emb1"], np.float32)   # [L, 3, H]
    e0x = np.repeat(ee0, 3, axis=1)                      # [L, 18, H]
    e1x = np.tile(ee1, (1, 6, 1))                        # [L, 18, H]
    beta = np.asarray(inputs["beta"], np.float32)        # [L]
    colv = np.zeros((P, 3 * LAYERS), np.float32)
    for l in range(LAYERS):
        colv[:, 3 * l + 0] = np.asarray(inputs["bn_gamma"])[l]
        colv[:, 3 * l + 1] = np.asarray(inputs["bn_beta"])[l]
        colv[:, 3 * l + 2] = beta[l]
    mlpb_rows = np.asarray(inputs["mlp_b"], np.float32)      # [L, H]
    bout_rep = np.tile(np.asarray(inputs["b_out"], np.float32)[None, :],
                       (NGRAPH, 1))                          # [16, 14]
    shared = dict(
        wn0=np.asarray(inputs["W_node0"], np.float32),
        wn1=np.asarray(inputs["W_node1"], np.float32),
        e0x=e0x.reshape(LAYERS * 18, H), e1x=e1x.reshape(LAYERS * 18, H),
        colv=colv,
        mlpw=np.asarray(inputs["mlp_W"], np.float32).reshape(LAYERS * H, H),
        mlpb=mlpb_rows,
        wout=np.asarray(inputs["W_out"], np.float32),
        bout=bout_rep,
    )
    for d in per_core:
        d.update(shared)
    return m, per_core


def _build(m):
    NPC, nblk = m.NPC, m.nblk
    NTmax = int(m.NT.max())
    nc = bacc.Bacc("TRN2", target_bir_lowering=False, debug=False,
                   num_devices=N_CORES)

    inp = {}
    def di(name, shape, dtype=f32):
        inp[name] = nc.dram_tensor(name, list(shape), dtype, kind="ExternalInput")
        return inp[name]

    di("gidx", [P, m.IXW], mybir.dt.int16)
    di("gcmb", [P, m.CXW], mybir.dt.int16)
    di("ind", [P * m.Wtot], bf16)
    di("oh0", [m.V0, NPC]); di("oh1", [m.V1, NPC])
    di("pmat", [nblk * P, NGRAPH])
    di("wn0", [m.V0, H]); di("wn1", [m.V1, H])
    di("e0x", [LAYERS * 18, H]); di("e1x", [LAYERS * 18, H])
    di("colv", [P, 3 * LAYERS])
    di("mlpw", [LAYERS * H, H]); di("mlpb", [LAYERS, H])
    di("wout", [H, OUT_DIM]); di("bout", [NGRAPH, OUT_DIM])
    out_t = nc.dram_tensor("out", [NGRAPH, OUT_DIM], f32, kind="ExternalOutput")
    dbg = {}
    if DEBUG:
        for nm in ("dbg_hv0", "dbg_hv1", "dbg_agg", "dbg_hvl0"):
            dbg[nm] = nc.dram_tensor(nm, [P, NPC], f32, kind="ExternalOutput")

    table = nc.dram_tensor("table", [m.N, H], bf16, kind="Internal",
                           addr_space="Shared")
    aginb = nc.dram_tensor("aginb", [NPC, H], bf16, kind="Internal")
    eetabs = [nc.dram_tensor(f"eetab{l}", [18, H], bf16, kind="Internal")
              for l in range(LAYERS)]
    arin = nc.dram_tensor("arin", [P, 2], f32, kind="Internal")
    arout = nc.dram_tensor("arout", [P, 2], f32, kind="Internal", addr_space="Shared")
    hgin = nc.dram_tensor("hgin", [NGRAPH, H + 1], f32, kind="Internal")
    hgout = nc.dram_tensor("hgout", [NGRAPH, H + 1], f32, kind="Internal",
                           addr_space="Shared")
    RG = [list(range(N_CORES))]

    NPC_pad = nblk * P

    with tile.TileContext(nc) as tc:
        with tc.tile_pool(name="persist", bufs=1) as pp, \
             tc.tile_pool(name="work", bufs=3) as wp, \
             tc.tile_pool(name="edge", bufs=2) as ep, \
             tc.tile_pool(name="small", bufs=2) as sp, \
             tc.tile_pool(name="ps", bufs=2, space="PSUM") as psp:

            # ---------------- constants ----------------
            ident = pp.tile([P, P], f32, tag="ident")
            make_identity(nc, ident[:])
            colv_sb = pp.tile([P, 3 * LAYERS], f32, tag="colv")
            nc.sync.dma_start(colv_sb[:], inp["colv"].ap())
            wn0_sb = pp.tile([m.V0, H], f32, tag="wn0")
            nc.sync.dma_start(wn0_sb[:], inp["wn0"].ap())
            wn1_sb = pp.tile([m.V1, H], f32, tag="wn1")
            nc.sync.dma_start(wn1_sb[:], inp["wn1"].ap())
            wout_sb = pp.tile([H, OUT_DIM], f32, tag="wout")
            nc.sync.dma_start(wout_sb[:], inp["wout"].ap())
            bout_sb = pp.tile([NGRAPH, OUT_DIM], f32, tag="bout")
            nc.sync.dma_start(bout_sb[:], inp["bout"].ap())
            ones_row = pp.tile([1, 512], f32, tag="ones")
            nc.vector.memset(ones_row[:], 1.0)
            onecol = pp.tile([P, 1], f32, tag="onecol")
            nc.vector.memset(onecol[:], 1.0)

            # persistent node state [H, NPC]
            hv = pp.tile([P, NPC], f32, tag="hv")
            hv1 = pp.tile([P, NPC], f32, tag="hv1")
            agg = pp.tile([P, NPC], f32, tag="agg")
            stage = pp.tile([P, NPC_pad], bf16, tag="stage")

            # ---------------- embedding ----------------
            nch = (NPC + 511) // 512
            for j in range(nch):
                w = min(512, NPC - j * 512)
                o0 = wp.tile([m.V0, 512], f32, tag="oh0")
                nc.sync.dma_start(o0[:, :w], inp["oh0"].ap()[:, j * 512 : j * 512 + w])
                o1 = wp.tile([m.V1, 512], f32, tag="oh1")
                nc.sync.dma_start(o1[:, :w], inp["oh1"].ap()[:, j * 512 : j * 512 + w])
                ps = psp.tile([P, 512], f32, tag="mm")
                nc.tensor.matmul(ps[:, :w], wn0_sb[:], o0[:, :w],
                                 start=True, stop=False)
                nc.tensor.matmul(ps[:, :w], wn1_sb[:], o1[:, :w],
                                 start=False, stop=True)
                nc.scalar.activation(hv[:, j * 512 : j * 512 + w], ps[:, :w],
                                     mybir.ActivationFunctionType.Copy)

            if DEBUG:
                nc.sync.dma_start(dbg["dbg_hv0"].ap(), hv[:])

            # ---------------- layers ----------------
            for l in range(LAYERS):
                gcol = colv_sb[:, 3 * l + 0 : 3 * l + 1]
                bcol = colv_sb[:, 3 * l + 1 : 3 * l + 2]
                betac = colv_sb[:, 3 * l + 2 : 3 * l + 3]

                # BN stats (partial) + AllReduce
                S = sp.tile([P, 1], f32, tag="S")
                nc.vector.reduce_sum(S[:], hv[:], axis=mybir.AxisListType.X)
                Q = sp.tile([P, 1], f32, tag="Q")
                nc.scalar.activation(stage[:, :NPC], hv[:],
                                     mybir.ActivationFunctionType.Square,
                                     accum_out=Q[:])
                sq = sp.tile([P, 2], f32, tag="sq")
                nc.vector.tensor_copy(sq[:, 0:1], S[:])
                nc.vector.tensor_copy(sq[:, 1:2], Q[:])
                nc.sync.dma_start(arin.ap(), sq[:])
                if "coll" not in ABLATE:
                    nc.gpsimd.collective_compute(
                        "AllReduce", mybir.AluOpType.add, replica_groups=RG,
                        ins=[arin.ap()], outs=[arout.ap()])
                sqg = sp.tile([P, 2], f32, tag="sqg")
                nc.sync.dma_start(sqg[:], arout.ap())

                mean = sp.tile([P, 1], f32, tag="mean")
                nc.vector.tensor_scalar_mul(mean[:], sqg[:, 0:1], 1.0 / m.N)
                var = sp.tile([P, 1], f32, tag="var")
                nc.vector.tensor_scalar_mul(var[:], sqg[:, 1:2], 1.0 / m.N)
                msq = sp.tile([P, 1], f32, tag="msq")
                nc.vector.tensor_mul(msq[:], mean[:], mean[:])
                nc.vector.tensor_tensor(var[:], var[:], msq[:],
                                        op=mybir.AluOpType.subtract)
                nc.vector.tensor_scalar_add(var[:], var[:], BN_EPS)
                rv = sp.tile([P, 1], f32, tag="rv")
                nc.vector.reciprocal(rv[:], var[:])
                rsq = sp.tile([P, 1], f32, tag="rsq")
                nc.scalar.activation(rsq[:], rv[:],
                                     mybir.ActivationFunctionType.Sqrt)
                sc = sp.tile([P, 1], f32, tag="sc")
                nc.vector.tensor_mul(sc[:], gcol, rsq[:])
                tsh = sp.tile([P, 1], f32, tag="tsh")
                nc.vector.tensor_mul(tsh[:], mean[:], sc[:])
                nc.vector.tensor_tensor(tsh[:], bcol, tsh[:],
                                        op=mybir.AluOpType.subtract)
                ebias = sp.tile([P, 1], f32, tag="ebias")
                nc.vector.tensor_scalar_mul(ebias[:], betac, GEN_EPS)
                bI = sp.tile([P, P], f32, tag="bI")
                nc.vector.tensor_scalar(bI[:], ident[:], betac, None,
                                        op0=mybir.AluOpType.mult)

                # hv1 = relu(sc*hv + tsh)
                nc.scalar.activation(hv1[:], hv[:],
                                     mybir.ActivationFunctionType.Relu,
                                     bias=tsh[:], scale=sc[:])

                # transpose beta*hv1 into stage, then DMA to aginb, AllGather
                for b in range(nblk):
                    w = min(P, NPC - b * P)
                    pst = psp.tile([P, P], f32, tag="tr")
                    nc.tensor.matmul(pst[:w, :], hv1[:, b * P : b * P + w],
                                     bI[:], start=True, stop=True)
                    nc.scalar.activation(stage[:w, b * P : (b + 1) * P],
                                         pst[:w, :],
                                         mybir.ActivationFunctionType.Copy)
                nfull = NPC // P
                ag_ap = aginb.ap()[: nfull * P, :].rearrange(
                    "(b p) h -> p b h", p=P)
                nc.sync.dma_start(ag_ap, stage[:, : nfull * P])
                if NPC % P:
                    w = NPC % P
                    nc.sync.dma_start(aginb.ap()[nfull * P :, :],
                                      stage[:w, nfull * P : nfull * P + P])
                if "coll" not in ABLATE:
                    nc.gpsimd.collective_compute(
                        "AllGather", mybir.AluOpType.bypass, replica_groups=RG,
                        ins=[aginb.ap()], outs=[table.ap()])

                # edge-embedding table: beta * (e0 + e1) -> eetabs[l]
                e0 = sp.tile([18, H], f32, tag="e0")
                nc.sync.dma_start(e0[:], inp["e0x"].ap()[l * 18:(l + 1) * 18, :])
                e1 = sp.tile([18, H], f32, tag="e1")
                nc.sync.dma_start(e1[:], inp["e1x"].ap()[l * 18:(l + 1) * 18, :])
                eet = sp.tile([18, H], f32, tag="eet")
                nc.vector.tensor_add(eet[:], e0[:], e1[:])
                eetb = sp.tile([18, H], bf16, tag="eetb")
                nc.vector.tensor_scalar(eetb[:], eet[:], betac[0:18, :], None,
                                        op0=mybir.AluOpType.mult)
                nc.sync.dma_start(eetabs[l].ap(), eetb[:])

                # ---------------- edge pass ----------------
                SPLIT = 32768
                hi_table = table.ap()[SPLIT:, :] if m.N > SPLIT else table.ap()
                for gi, grp in enumerate(m.groups):
                    TLO, THI = grp["tlo"], grp["thi"]
                    NTg = TLO + THI
                    o = m.ixoff[gi]
                    ixl = ep.tile([P, m.GT * 8], mybir.dt.int16, tag="ixl")
                    nc.sync.dma_start(ixl[:, : TLO * 8],
                                      inp["gidx"].ap()[:, o : o + TLO * 8])
                    ixh = ep.tile([P, m.GT * 8], mybir.dt.int16, tag="ixh")
                    nc.sync.dma_start(
                        ixh[:, : THI * 8],
                        inp["gidx"].ap()[:, o + TLO * 8 : o + NTg * 8])
                    oc = m.cxoff[gi]
                    cmbt = ep.tile([P, m.GT * 8], mybir.dt.int16, tag="cmb")
                    nc.sync.dma_start(cmbt[:, : NTg * 8],
                                      inp["gcmb"].ap()[:, oc : oc + NTg * 8])

                    g = ep.tile([P, m.GT * P], bf16, tag="g")
                    ge = ep.tile([P, m.GT * P], bf16, tag="ge")
                    if "gather" not in ABLATE:
                     nc.gpsimd.dma_gather(
                        out_ap=g[:, : TLO * P].rearrange(
                            "p (n h) -> p n h", n=TLO),
                        in_ap=table.ap(),
                        idxs_ap=ixl[:, : TLO * 8],
                        num_idxs=TLO * P, num_idxs_reg=TLO * P, elem_size=H,
                        single_packet=False)
                     nc.gpsimd.dma_gather(
                        out_ap=g[:, TLO * P : NTg * P].rearrange(
                            "p (n h) -> p n h", n=THI),
                        in_ap=hi_table,
                        idxs_ap=ixh[:, : THI * 8],
                        num_idxs=THI * P, num_idxs_reg=THI * P, elem_size=H,
                        single_packet=False)
                     nc.gpsimd.dma_gather(
                        out_ap=ge[:, : NTg * P].rearrange(
                            "p (n h) -> p n h", n=NTg),
                        in_ap=eetabs[l].ap(),
                        idxs_ap=cmbt[:, : NTg * 8],
                        num_idxs=NTg * P, num_idxs_reg=NTg * P, elem_size=H,
                        single_packet=False)

                    EW = NTg * P
                    # t = hsrc + ee (in place into g); m = relu(t) -> ge;
                    # e = exp(m) -> g; me = m*e -> ge
                    if "elem" not in ABLATE:
                        nc.vector.tensor_add(g[:, :EW], g[:, :EW], ge[:, :EW])
                        nc.scalar.activation(ge[:, :EW], g[:, :EW],
                                             mybir.ActivationFunctionType.Relu)
                        nc.scalar.activation(g[:, :EW], ge[:, :EW],
                                             mybir.ActivationFunctionType.Exp)
                        nc.vector.tensor_mul(ge[:, :EW], ge[:, :EW], g[:, :EW])

                    for b in grp["blocks"]:
                        if "segmm" in ABLATE:
                            break
                        nlo_b, nhi_b = int(m.nlo_t[b]), int(m.nhi_t[b])
                        ntot = nlo_b + nhi_b
                        dw = m.dwin[b]
                        Wb = sum(w for (_, w) in m.covers[b])
                        ind = ep.tile([P, m.Wmax], bf16, tag="ind")
                        nc.sync.dma_start(
                            ind[:, :Wb],
                            inp["ind"].ap()[m.woff[b] * P : (m.woff[b] + Wb) * P]
                            .rearrange("(p n) -> p n", p=P))
                        psE = psp.tile([P, P], f32, tag="segE")
                        psM = psp.tile([P, P], f32, tag="segM")
                        wo = 0
                        for t in range(ntot):
                            lo, w = m.covers[b][t]
                            if t < nlo_b:
                                gt = grp["lo_toff"][b] + t
                            else:
                                gt = TLO + grp["hi_toff"][b] + (t - nlo_b)
                            lhsT = ind[:, wo : wo + w]
                            nc.tensor.matmul(psE[lo : lo + w, :], lhsT,
                                             g[:, gt * P : (gt + 1) * P],
                                             start=(t == 0), stop=(t == ntot - 1),
                                             skip_group_check=True)
                            nc.tensor.matmul(psM[lo : lo + w, :], lhsT,
                                             ge[:, gt * P : (gt + 1) * P],
                                             start=(t == 0), stop=(t == ntot - 1),
                                             skip_group_check=True)
                            wo += w

                        den = ep.tile([P, P], f32, tag="den")
                        nc.scalar.activation(den[:dw, :], psE[:dw, :],
                                             mybir.ActivationFunctionType.Copy,
                                             bias=DEN_TINY, scale=betac[0:dw, :])
                        nc.vector.reciprocal(den[:dw, :], den[:dw, :])
                        adh = ep.tile([P, P], f32, tag="adh")
                        nc.vector.tensor_mul(adh[:dw, :], psM[:dw, :],
                                             den[:dw, :])
                        pst = psp.tile([P, P], f32, tag="tr")
                        nc.tensor.matmul(pst[:, :dw], adh[:dw, :],
                                         ident[:dw, :dw], start=True, stop=True)
                        nc.scalar.activation(agg[:, b * P : b * P + dw],
                                             pst[:, :dw],
                                             mybir.ActivationFunctionType.Copy)

                if DEBUG and l == 0:
                    nc.sync.dma_start(dbg["dbg_hv1"].ap(), hv1[:])
                    nc.sync.dma_start(dbg["dbg_agg"].ap(), agg[:])

                # X = hv1 + agg  (into agg)
                nc.vector.tensor_add(agg[:], agg[:], hv1[:])

                # MLP + bias + skip
                mw = wp.tile([H, H], f32, tag="mw")
                nc.sync.dma_start(mw[:], inp["mlpw"].ap()[l * H:(l + 1) * H, :])
                mb = wp.tile([1, H], f32, tag="mb")
                nc.sync.dma_start(mb[:], inp["mlpb"].ap()[l : l + 1, :])
                for j in range(nch):
                    w = min(512, NPC - j * 512)
                    ps = psp.tile([P, 512], f32, tag="mm")
                    nc.tensor.matmul(ps[:, :w], mb[:], ones_row[:, :w],
                                     start=True, stop=False)
                    nc.tensor.matmul(ps[:, :w], mw[:],
                                     agg[:, j * 512 : j * 512 + w],
                                     start=False, stop=True)
                    nc.vector.tensor_add(hv[:, j * 512 : j * 512 + w],
                                         hv[:, j * 512 : j * 512 + w],
                                         ps[:, :w])

            if DEBUG:
                nc.sync.dma_start(dbg["dbg_hvl0"].ap(), hv[:])

            # ---------------- pooling + output ----------------
            psg = psp.tile([NGRAPH, H], f32, tag="segE")
            psc = psp.tile([NGRAPH, 1], f32, tag="segM")
            for b in range(nblk):
                w = min(P, NPC - b * P)
                pst = psp.tile([P, P], f32, tag="tr")
                nc.tensor.matmul(pst[:w, :], hv[:, b * P : b * P + w],
                                 ident[:], start=True, stop=True)
                xs = wp.tile([P, P], f32, tag="xs")
                nc.scalar.activation(xs[:w, :], pst[:w, :],
                                     mybir.ActivationFunctionType.Copy)
                pm = wp.tile([P, NGRAPH], f32, tag="pm")
                nc.sync.dma_start(pm[:], inp["pmat"].ap()[b * P:(b + 1) * P, :])
                nc.tensor.matmul(psg[:], pm[:w, :], xs[:w, :],
                                 start=(b == 0), stop=(b == nblk - 1))
                nc.tensor.matmul(psc[:], pm[:w, :], onecol[:w, :],
                                 start=(b == 0), stop=(b == nblk - 1))
            hgc = sp.tile([NGRAPH, H + 1], f32, tag="hgc")
            nc.scalar.activation(hgc[:, :H], psg[:],
                                 mybir.ActivationFunctionType.Copy)
            nc.scalar.activation(hgc[:, H : H + 1], psc[:],
                                 mybir.ActivationFunctionType.Copy)
            nc.sync.dma_start(hgin.ap(), hgc[:])
            nc.gpsimd.collective_compute(
                "AllReduce", mybir.AluOpType.add, replica_groups=RG,
                ins=[hgin.ap()], outs=[hgout.ap()])
            hgg = sp.tile([NGRAPH, H + 1], f32, tag="hgg")
            nc.sync.dma_start(hgg[:], hgout.ap())
            rcnt = sp.tile([NGRAPH, 1], f32, tag="rcnt")
            nc.vector.reciprocal(rcnt[:], hgg[:, H : H + 1])
            hgs = sp.tile([NGRAPH, H], f32, tag="hgs")
            nc.vector.tensor_scalar(hgs[:], hgg[:, :H], rcnt[:], None,
                                    op0=mybir.AluOpType.mult)
            pst = psp.tile([P, P], f32, tag="tr")
            nc.tensor.matmul(pst[:, :NGRAPH], hgs[:], ident[:NGRAPH, :NGRAPH],
                             start=True, stop=True)
            hgT = sp.tile([P, NGRAPH], f32, tag="hgT")
            nc.scalar.activation(hgT[:], pst[:, :NGRAPH],
                                 mybir.ActivationFunctionType.Copy)
            pso = psp.tile([NGRAPH, OUT_DIM], f32, tag="mm")
            nc.tensor.matmul(pso[:], hgT[:], wout_sb[:], start=True, stop=True)
            ot = sp.tile([NGRAPH, OUT_DIM], f32, tag="ot")
            nc.vector.tensor_add(ot[:], bout_sb[:], pso[:])
            nc.sync.dma_start(out_t.ap(), ot[:])

    nc.compile()
    return nc


def run(inputs, trace=False):
    m, per_core = _preprocess(inputs)
    nc = _build(m)
    res = bass_utils.run_bass_kernel_spmd(
        nc, per_core, core_ids=list(range(N_CORES)), trace=trace)
    return np.asarray(res.results[0]["out"], np.float32), res


def kernel(**inputs) -> np.ndarray:
    out, _ = run(inputs)
    return out


if __name__ == "__main__":
    import reference as R
    inputs = {k: np.asarray(v) for k, v in R.setup_inputs().items()}
    out = kernel(**inputs)
    print(out.shape, out.dtype)



# revision 11
# speedup vs baseline: 1.1463x; 1.1463x over previous
"""Trainium2 Bass kernel for a 3-layer GENConv-style GNN (DGCN) on 8 NeuronCores.

Strategy (graph-partition data parallel):
  - Nodes are split contiguously across 8 cores (6250 nodes/core); each core owns
    all edges whose *destination* lies in its node range.
  - Node state hv lives in SBUF as [H=128 partitions, nodes] per core.
  - Per layer: tiny AllReduce of BatchNorm statistics -> BN+ReLU in one ScalarE
    activation pass -> transpose hv1 tiles (scaled by beta via a beta*I identity)
    into a DRAM shard -> AllGather into a per-core full gather table [N, H] ->
    edge pass: indirect-DMA gather of hv1[src] rows (two gathers over the
    int16-index table halves, rotated across 4 SWDGE queues), edge-embedding
    rows added via TensorE matmuls against a host-packed 18-row one-hot stream,
    ReLU and exp on ScalarE into an interleaved [e|me] layout, and segment sums
    via a single TensorE matmul per edge tile against 0/1 indicator tiles that
    are generated on-chip (broadcasted is_equal against an iota row) ->
    agg = (sum m*e) / (beta * sum e) -> MLP matmul (+bias via a rank-1 matmul)
    + skip.
  - Softmax max-subtraction is skipped (z = beta*m is bounded, exp can't
    overflow; result is shift-invariant); empty segments guarded with +1e-30.
  - Final average pooling via a host-built 0/1 pooling matrix on TensorE; the
    per-core partial [16, H+1] (sums + counts) is AllReduced, and every core
    computes the output Linear.

The program is SPMD (one instruction stream for all 8 cores), so all
data-dependent tiling metadata (tiles per dst-block, indicator widths) is
uniformized across cores by taking per-(block,tile) maxima; the actual indices
and indicator contents are per-core input data.
"""

import sys

sys.path.insert(0, "/opt/trn_rl_repo")

import numpy as np
import ml_dtypes

import concourse.bass as bass
import concourse.bacc as bacc
import concourse.tile as tile
import concourse.mybir as mybir
from concourse import bass_utils
from concourse.masks import make_identity

f32 = mybir.dt.float32
bf16 = mybir.dt.bfloat16
i32 = mybir.dt.int32

N_CORES = 8
H = 128
P = 128
OUT_DIM = 14
NGRAPH = 16
LAYERS = 3
BN_EPS = 1e-5
GEN_EPS = 1e-7
DEN_TINY = 1e-30
DEBUG = False
ABLATE = set()  # timing ablations: "gather","elem","segmm","coll"
NUM_QUEUES = 4       # SWDGE queues (1-4)
GT = 28              # tiles per gather group
TB = 4               # blocks per batched transpose


class Meta:
    pass


def _preprocess(inputs):
    """Host-side index preprocessing + per-core input shards."""
    m = Meta()
    src = np.asarray(inputs["edge_src"], np.int64)
    dst = np.asarray(inputs["edge_dst"], np.int64)
    combo = (np.asarray(inputs["edge_feat0"], np.int64) * 3
             + np.asarray(inputs["edge_feat1"], np.int64))
    nf0 = np.asarray(inputs["node_feat0"], np.int64)
    nf1 = np.asarray(inputs["node_feat1"], np.int64)
    gids = np.asarray(inputs["graph_ids"], np.int64)

    N = nf0.shape[0]
    V0 = np.asarray(inputs["W_node0"]).shape[0]
    V1 = np.asarray(inputs["W_node1"]).shape[0]
    NPC = N // N_CORES
    nblk = (NPC + P - 1) // P
    m.N, m.NPC, m.nblk = N, NPC, nblk
    m.V0, m.V1 = V0, V1

    order = np.argsort(dst, kind="stable")
    dsts, srcs, combos = dst[order], src[order], combo[order]

    SPLIT = 32768  # int16 gather-index limit; table accessed as two halves
    rng = {}
    for c in range(N_CORES):
        for b in range(nblk):
            lo = c * NPC + b * P
            hi = min(c * NPC + min(NPC, (b + 1) * P), (c + 1) * NPC)
            i0 = np.searchsorted(dsts, lo, "left")
            i1 = np.searchsorted(dsts, hi, "left")
            rng[(c, b)] = (i0, i1)
    m.dwin = [min(P, NPC - b * P) for b in range(nblk)]

    # per-(core, block) edges reordered: (src < SPLIT, by dst), then (src >= SPLIT, by dst)
    # pad each half to a cross-core-uniform number of 128-edge tiles.
    core_blk = {}
    nlo_t = np.ones(nblk, dtype=np.int64)
    nhi_t = np.ones(nblk, dtype=np.int64)
    for c in range(N_CORES):
        for b in range(nblk):
            i0, i1 = rng[(c, b)]
            s_b = srcs[i0:i1]
            d_b = dsts[i0:i1] - (c * NPC + b * P)
            c_b = combos[i0:i1]
            is_hi = s_b >= SPLIT
            lo_sel = np.argsort(d_b[~is_hi], kind="stable")
            hi_sel = np.argsort(d_b[is_hi], kind="stable")
            parts = ((s_b[~is_hi][lo_sel], d_b[~is_hi][lo_sel], c_b[~is_hi][lo_sel]),
                     (s_b[is_hi][hi_sel] - SPLIT, d_b[is_hi][hi_sel], c_b[is_hi][hi_sel]))
            core_blk[(c, b)] = parts
            nlo_t[b] = max(nlo_t[b], (len(parts[0][0]) + P - 1) // P)
            nhi_t[b] = max(nhi_t[b], (len(parts[1][0]) + P - 1) // P)
    m.nlo_t, m.nhi_t = nlo_t, nhi_t
    NT = nlo_t + nhi_t
    m.NT = NT

    # uniform covers per (b, run, t): cross-core dst spans; block-first/last
    # tiles full-width (PSUM zero/close discipline); middles quadrant-aligned.
    covers = []
    for b in range(nblk):
        spans = []
        for run, ntr in ((0, int(nlo_t[b])), (1, int(nhi_t[b]))):
            for t in range(ntr):
                lo_u, hi_u = 1 << 30, 0
                for c in range(N_CORES):
                    dl = core_blk[(c, b)][run][1]
                    ch = dl[t * P:(t + 1) * P]
                    if len(ch):
                        lo_u = min(lo_u, int(ch.min()))
                        hi_u = max(hi_u, int(ch.max() + 1))
                spans.append((lo_u, hi_u))
        cv = []
        ntot = int(NT[b])
        for t in range(ntot):
            lo_u, hi_u = spans[t]
            if t == 0 or t == ntot - 1:
                cv.append((0, m.dwin[b]))
            else:
                lo = min(lo_u, m.dwin[b] - 1)
                hi = max(min(hi_u, m.dwin[b]), lo + 1)
                a32 = (lo // 32) * 32
                a64 = (lo // 64) * 64
                if hi <= a32 + 32 and a32 in (0, 32, 64):
                    cv.append((a32, 32))
                elif hi <= a64 + 64:
                    cv.append((a64, 64))
                else:
                    cv.append((0, P))
        covers.append(cv)
    m.covers = covers
    m.T = int(NT.sum())

    # group consecutive blocks for batched gathers
    groups, cur = [], None
    for b in range(nblk):
        if cur is None or cur["nt"] + int(NT[b]) > GT:
            cur = dict(blocks=[], nt=0, tlo=0, thi=0)
            groups.append(cur)
        cur["blocks"].append(b)
        cur["nt"] += int(NT[b])
        cur["tlo"] += int(nlo_t[b])
        cur["thi"] += int(nhi_t[b])
    for grp in groups:
        lt = ht = 0
        grp["lo_toff"], grp["hi_toff"] = {}, {}
        for b in grp["blocks"]:
            grp["lo_toff"][b] = lt; lt += int(nlo_t[b])
            grp["hi_toff"][b] = ht; ht += int(nhi_t[b])
    m.groups = groups
    # flat offsets: ixoff (idx stream, 16-wrapped cols), txoff (tile cols)
    ixoff, txoff = [], []
    ax = tx = 0
    for grp in groups:
        ixoff.append(ax)
        txoff.append(tx)
        ax += (grp["tlo"] + grp["thi"]) * 8
        tx += grp["tlo"] + grp["thi"]
    m.ixoff, m.IXW = ixoff, ax
    m.txoff = txoff
    assert tx == m.T

    def wrap16(seq):
        sq = seq.astype(np.int16).reshape(-1, 16).T        # [16, S]
        return np.tile(sq, (8, 1))                          # [128, S]

    # --- per-core data arrays ---
    per_core = []
    for c in range(N_CORES):
        gidx = np.zeros((P, m.IXW), np.int16)
        dloc = np.full((P, m.T), -1.0, np.float32)
        ohc = np.zeros((18, m.T * P), ml_dtypes.bfloat16)
        blk_streams = {}
        for b in range(nblk):
            (ls, ld, lc), (hs, hd, hc) = core_blk[(c, b)]
            streams = []
            for (s_, d_, c_), ntr in (((ls, ld, lc), int(nlo_t[b])),
                                      ((hs, hd, hc), int(nhi_t[b]))):
                npad = ntr * P - len(s_)
                streams.append((
                    np.concatenate([s_, np.zeros(npad, np.int64)]),
                    np.concatenate([d_, np.full(npad, -1, np.int64)]),
                    np.concatenate([c_, np.zeros(npad, np.int64)])))
            blk_streams[b] = streams
        # idx streams per group + dloc + combo one-hot (all in g-tile order)
        for gi, grp in enumerate(groups):
            lo_seq = np.concatenate(
                [blk_streams[b][0][0] for b in grp["blocks"]])
            hi_seq = np.concatenate(
                [blk_streams[b][1][0] for b in grp["blocks"]])
            o = m.ixoff[gi]
            gidx[:, o : o + grp["tlo"] * 8] = wrap16(lo_seq)
            gidx[:, o + grp["tlo"] * 8 : o + (grp["tlo"] + grp["thi"]) * 8] = \
                wrap16(hi_seq)
            d_seq = np.concatenate(
                [blk_streams[b][0][1] for b in grp["blocks"]]
                + [blk_streams[b][1][1] for b in grp["blocks"]])
            c_seq = np.concatenate(
                [blk_streams[b][0][2] for b in grp["blocks"]]
                + [blk_streams[b][1][2] for b in grp["blocks"]])
            # per-tile cover lo offsets, in the same tile order
            lo_t = []
            for b in grp["blocks"]:
                lo_t.extend(cv[0] for cv in covers[b][: int(nlo_t[b])])
            for b in grp["blocks"]:
                lo_t.extend(
                    cv[0] for cv in covers[b][int(nlo_t[b]) : int(NT[b])])
            to = m.txoff[gi]
            ntg = grp["tlo"] + grp["thi"]
            dv = d_seq.reshape(ntg, P)
            lt = np.asarray(lo_t, np.int64).reshape(ntg, 1)
            dl = np.where(dv >= 0, dv - lt, -1).astype(np.float32)
            dloc[:, to : to + ntg] = dl.T
            ohc[c_seq, np.arange(to * P, (to + ntg) * P)] = 1.0

        nlo, nhi = c * NPC, (c + 1) * NPC
        oh0 = np.zeros((V0, NPC), np.float32)
        oh0[nf0[nlo:nhi], np.arange(NPC)] = 1.0
        oh1 = np.zeros((V1, NPC), np.float32)
        oh1[nf1[nlo:nhi], np.arange(NPC)] = 1.0
        pmat = np.zeros((nblk * P, NGRAPH), np.float32)
        g_c = gids[nlo:nhi]
        for b in range(nblk):
            w = min(P, NPC - b * P)
            pmat[b * P : b * P + w, :][np.arange(w), g_c[b * P : b * P + w]] = 1.0
        per_core.append(dict(
            gidx=gidx, dloc=dloc, ohc=ohc,
            oh0=oh0, oh1=oh1, pmat=pmat,
        ))

    # replicated parameter tensors
    ee0 = np.asarray(inputs["edge_emb0"], np.float32)   # [L, 6, H]
    ee1 = np.asarray(inputs["edge_emb1"], np.float32)   # [L, 3, H]
    e0x = np.repeat(ee0, 3, axis=1)                      # [L, 18, H]
    e1x = np.tile(ee1, (1, 6, 1))                        # [L, 18, H]
    beta = np.asarray(inputs["beta"], np.float32)        # [L]
    colv = np.zeros((P, 3 * LAYERS), np.float32)
    for l in range(LAYERS):
        colv[:, 3 * l + 0] = np.asarray(inputs["bn_gamma"])[l]
        colv[:, 3 * l + 1] = np.asarray(inputs["bn_beta"])[l]
        colv[:, 3 * l + 2] = beta[l]
    mlpb_rows = np.asarray(inputs["mlp_b"], np.float32)      # [L, H]
    bout_rep = np.tile(np.asarray(inputs["b_out"], np.float32)[None, :],
                       (NGRAPH, 1))                          # [16, 14]
    shared = dict(
        wn0=np.asarray(inputs["W_node0"], np.float32),
        wn1=np.asarray(inputs["W_node1"], np.float32),
        e0x=e0x.reshape(LAYERS * 18, H), e1x=e1x.reshape(LAYERS * 18, H),
        colv=colv,
        mlpw=np.asarray(inputs["mlp_W"], np.float32).reshape(LAYERS * H, H),
        mlpb=mlpb_rows,
        wout=np.asarray(inputs["W_out"], np.float32),
        bout=bout_rep,
    )
    for d in per_core:
        d.update(shared)
    return m, per_core


def _build(m):
    NPC, nblk = m.NPC, m.nblk
    nc = bacc.Bacc("TRN2", target_bir_lowering=False, debug=False,
                   num_devices=N_CORES, num_swdge_queues=NUM_QUEUES)

    inp = {}
    def di(name, shape, dtype=f32):
        inp[name] = nc.dram_tensor(name, list(shape), dtype, kind="ExternalInput")
        return inp[name]

    di("gidx", [P, m.IXW], mybir.dt.int16)
    di("dloc", [P, m.T])
    di("ohc", [18, m.T * P], bf16)
    di("oh0", [m.V0, NPC]); di("oh1", [m.V1, NPC])
    di("pmat", [nblk * P, NGRAPH])
    di("wn0", [m.V0, H]); di("wn1", [m.V1, H])
    di("e0x", [LAYERS * 18, H]); di("e1x", [LAYERS * 18, H])
    di("colv", [P, 3 * LAYERS])
    di("mlpw", [LAYERS * H, H]); di("mlpb", [LAYERS, H])
    di("wout", [H, OUT_DIM]); di("bout", [NGRAPH, OUT_DIM])
    out_t = nc.dram_tensor("out", [NGRAPH, OUT_DIM], f32, kind="ExternalOutput")
    dbg = {}
    if DEBUG:
        for nm in ("dbg_hv0", "dbg_hv1", "dbg_agg", "dbg_hvl0"):
            dbg[nm] = nc.dram_tensor(nm, [P, NPC], f32, kind="ExternalOutput")

    table = nc.dram_tensor("table", [m.N, H], bf16, kind="Internal",
                           addr_space="Shared")
    aginb = nc.dram_tensor("aginb", [NPC, H], bf16, kind="Internal")
    arin = nc.dram_tensor("arin", [P, 2], f32, kind="Internal")
    arout = nc.dram_tensor("arout", [P, 2], f32, kind="Internal", addr_space="Shared")
    hgin = nc.dram_tensor("hgin", [NGRAPH, H + 1], f32, kind="Internal")
    hgout = nc.dram_tensor("hgout", [NGRAPH, H + 1], f32, kind="Internal",
                           addr_space="Shared")
    RG = [list(range(N_CORES))]

    NPC_pad = nblk * P

    with tile.TileContext(nc) as tc:
        with tc.tile_pool(name="persist", bufs=1) as pp, \
             tc.tile_pool(name="work", bufs=3) as wp, \
             tc.tile_pool(name="edge", bufs=2) as ep, \
             tc.tile_pool(name="small", bufs=2) as sp, \
             tc.tile_pool(name="ps", bufs=2, space="PSUM") as psp, \
             tc.tile_pool(name="psee", bufs=2, space="PSUM") as pse:

            # ---------------- constants ----------------
            ident = pp.tile([P, P], f32, tag="ident")
            make_identity(nc, ident[:])
            iott = pp.tile([P, P], f32, tag="iott")
            nc.gpsimd.iota(iott[:], pattern=[[1, P]], base=0,
                           channel_multiplier=0,
                           allow_small_or_imprecise_dtypes=True)
            colv_sb = pp.tile([P, 3 * LAYERS], f32, tag="colv")
            nc.sync.dma_start(colv_sb[:], inp["colv"].ap())
            wn0_sb = pp.tile([m.V0, H], f32, tag="wn0")
            nc.sync.dma_start(wn0_sb[:], inp["wn0"].ap())
            wn1_sb = pp.tile([m.V1, H], f32, tag="wn1")
            nc.sync.dma_start(wn1_sb[:], inp["wn1"].ap())
            wout_sb = pp.tile([H, OUT_DIM], f32, tag="wout")
            nc.sync.dma_start(wout_sb[:], inp["wout"].ap())
            bout_sb = pp.tile([NGRAPH, OUT_DIM], f32, tag="bout")
            nc.sync.dma_start(bout_sb[:], inp["bout"].ap())
            ones_row = pp.tile([1, 512], f32, tag="ones")
            nc.vector.memset(ones_row[:], 1.0)
            onecol = pp.tile([P, 1], f32, tag="onecol")
            nc.vector.memset(onecol[:], 1.0)

            # persistent streams
            gidx_sb = pp.tile([P, m.IXW], mybir.dt.int16, tag="gidx")
            nc.sync.dma_start(gidx_sb[:], inp["gidx"].ap())
            dloc_sb = pp.tile([P, m.T], f32, tag="dloc")
            nc.sync.dma_start(dloc_sb[:], inp["dloc"].ap())

            # persistent node state [H, NPC]
            hv = pp.tile([P, NPC], f32, tag="hv")
            hv1 = pp.tile([P, NPC], f32, tag="hv1")
            agg = pp.tile([P, NPC], f32, tag="agg")
            stage = pp.tile([P, NPC_pad], bf16, tag="stage")

            # ---------------- embedding ----------------
            nch = (NPC + 511) // 512
            for j in range(nch):
                w = min(512, NPC - j * 512)
                o0 = wp.tile([m.V0, 512], f32, tag="oh0")
                nc.sync.dma_start(o0[:, :w], inp["oh0"].ap()[:, j * 512 : j * 512 + w])
                o1 = wp.tile([m.V1, 512], f32, tag="oh1")
                nc.sync.dma_start(o1[:, :w], inp["oh1"].ap()[:, j * 512 : j * 512 + w])
                ps = psp.tile([P, 512], f32, tag="mm")
                nc.tensor.matmul(ps[:, :w], wn0_sb[:], o0[:, :w],
                                 start=True, stop=False)
                nc.tensor.matmul(ps[:, :w], wn1_sb[:], o1[:, :w],
                                 start=False, stop=True)
                nc.scalar.activation(hv[:, j * 512 : j * 512 + w], ps[:, :w],
                                     mybir.ActivationFunctionType.Copy)

            if DEBUG:
                nc.sync.dma_start(dbg["dbg_hv0"].ap(), hv[:])

            # ---------------- layers ----------------
            for l in range(LAYERS):
                gcol = colv_sb[:, 3 * l + 0 : 3 * l + 1]
                bcol = colv_sb[:, 3 * l + 1 : 3 * l + 2]
                betac = colv_sb[:, 3 * l + 2 : 3 * l + 3]

                # BN stats (partial) + AllReduce
                S = sp.tile([P, 1], f32, tag="S")
                nc.vector.reduce_sum(S[:], hv[:], axis=mybir.AxisListType.X)
                Q = sp.tile([P, 1], f32, tag="Q")
                nc.scalar.activation(stage[:, :NPC], hv[:],
                                     mybir.ActivationFunctionType.Square,
                                     accum_out=Q[:])
                sq = sp.tile([P, 2], f32, tag="sq")
                nc.vector.tensor_copy(sq[:, 0:1], S[:])
                nc.vector.tensor_copy(sq[:, 1:2], Q[:])
                nc.sync.dma_start(arin.ap(), sq[:])
                if "coll" not in ABLATE:
                    nc.gpsimd.collective_compute(
                        "AllReduce", mybir.AluOpType.add, replica_groups=RG,
                        ins=[arin.ap()], outs=[arout.ap()])
                sqg = sp.tile([P, 2], f32, tag="sqg")
                nc.sync.dma_start(sqg[:], arout.ap() if "coll" not in ABLATE
                                  else arin.ap())

                mean = sp.tile([P, 1], f32, tag="mean")
                nc.vector.tensor_scalar_mul(mean[:], sqg[:, 0:1], 1.0 / m.N)
                var = sp.tile([P, 1], f32, tag="var")
                nc.vector.tensor_scalar_mul(var[:], sqg[:, 1:2], 1.0 / m.N)
                msq = sp.tile([P, 1], f32, tag="msq")
                nc.vector.tensor_mul(msq[:], mean[:], mean[:])
                nc.vector.tensor_tensor(var[:], var[:], msq[:],
                                        op=mybir.AluOpType.subtract)
                nc.vector.tensor_scalar_add(var[:], var[:], BN_EPS)
                rv = sp.tile([P, 1], f32, tag="rv")
                nc.vector.reciprocal(rv[:], var[:])
                rsq = sp.tile([P, 1], f32, tag="rsq")
                nc.scalar.activation(rsq[:], rv[:],
                                     mybir.ActivationFunctionType.Sqrt)
                sc = sp.tile([P, 1], f32, tag="sc")
                nc.vector.tensor_mul(sc[:], gcol, rsq[:])
                tsh = sp.tile([P, 1], f32, tag="tsh")
                nc.vector.tensor_mul(tsh[:], mean[:], sc[:])
                nc.vector.tensor_tensor(tsh[:], bcol, tsh[:],
                                        op=mybir.AluOpType.subtract)
                bI = sp.tile([P, P], f32, tag="bI")
                nc.vector.tensor_scalar(bI[:], ident[:], betac, None,
                                        op0=mybir.AluOpType.mult)

                # hv1 = relu(sc*hv + tsh)
                nc.scalar.activation(hv1[:], hv[:],
                                     mybir.ActivationFunctionType.Relu,
                                     bias=tsh[:], scale=sc[:])

                # transpose beta*hv1 into stage (batched TB blocks per PSUM
                # tile + single copy), then DMA to aginb, AllGather
                for b0 in range(0, nblk, TB):
                    bn = min(TB, nblk - b0)
                    pst = psp.tile([P, TB * P], f32, tag="tr")
                    for k in range(bn):
                        b = b0 + k
                        w = min(P, NPC - b * P)
                        nc.tensor.matmul(pst[:w, k * P : (k + 1) * P],
                                         hv1[:, b * P : b * P + w],
                                         bI[:], start=True, stop=True)
                    nc.scalar.activation(stage[:, b0 * P : (b0 + bn) * P],
                                         pst[:, : bn * P],
                                         mybir.ActivationFunctionType.Copy)
                nfull = NPC // P
                ag_ap = aginb.ap()[: nfull * P, :].rearrange(
                    "(b p) h -> p b h", p=P)
                nc.sync.dma_start(ag_ap, stage[:, : nfull * P])
                if NPC % P:
                    w = NPC % P
                    nc.sync.dma_start(aginb.ap()[nfull * P :, :],
                                      stage[:w, nfull * P : nfull * P + P])
                if "coll" not in ABLATE:
                    nc.gpsimd.collective_compute(
                        "AllGather", mybir.AluOpType.bypass, replica_groups=RG,
                        ins=[aginb.ap()], outs=[table.ap()])

                # edge-embedding table: beta * (e0 + e1) kept in SBUF (bf16)
                e0 = sp.tile([18, H], f32, tag="e0")
                nc.sync.dma_start(e0[:], inp["e0x"].ap()[l * 18:(l + 1) * 18, :])
                e1 = sp.tile([18, H], f32, tag="e1")
                nc.sync.dma_start(e1[:], inp["e1x"].ap()[l * 18:(l + 1) * 18, :])
                eet = sp.tile([18, H], f32, tag="eet")
                nc.vector.tensor_add(eet[:], e0[:], e1[:])
                eetb = sp.tile([18, H], bf16, tag="eetb")
                nc.vector.tensor_scalar(eetb[:], eet[:], betac[0:18, :], None,
                                        op0=mybir.AluOpType.mult)

                # ---------------- edge pass ----------------
                SPLIT = 32768
                hi_table = table.ap()[SPLIT:, :] if m.N > SPLIT else table.ap()
                for gi, grp in enumerate(m.groups):
                    TLO, THI = grp["tlo"], grp["thi"]
                    NTg = TLO + THI
                    EW = NTg * P
                    o = m.ixoff[gi]
                    to = m.txoff[gi]

                    g = ep.tile([P, GT * P], bf16, tag="g")
                    gme = ep.tile([P, GT * 2 * P], bf16, tag="gme")
                    if "gather" not in ABLATE:
                     nc.gpsimd.dma_gather(
                        out_ap=g[:, : TLO * P].rearrange(
                            "p (n h) -> p n h", n=TLO),
                        in_ap=table.ap(),
                        idxs_ap=gidx_sb[:, o : o + TLO * 8],
                        num_idxs=TLO * P, num_idxs_reg=TLO * P, elem_size=H,
                        single_packet=False,
                        queue_num=(2 * gi) % NUM_QUEUES)
                     nc.gpsimd.dma_gather(
                        out_ap=g[:, TLO * P : NTg * P].rearrange(
                            "p (n h) -> p n h", n=THI),
                        in_ap=hi_table,
                        idxs_ap=gidx_sb[:, o + TLO * 8 : o + NTg * 8],
                        num_idxs=THI * P, num_idxs_reg=THI * P, elem_size=H,
                        single_packet=False,
                        queue_num=(2 * gi + 1) % NUM_QUEUES)

                    # combo one-hot stream + indicator generation (on-chip)
                    ohg = ep.tile([18, GT * P], bf16, tag="ohg")
                    nc.sync.dma_start(ohg[:, :EW],
                                      inp["ohc"].ap()[:, to * P : to * P + EW])
                    ind = ep.tile([P, GT * P], bf16, tag="ind")
                    if "segmm" not in ABLATE:
                        nc.vector.tensor_tensor(
                            ind[:].rearrange("p (t j) -> p t j", t=GT)[:, :NTg, :],
                            dloc_sb[:, to : to + NTg]
                                .rearrange("p (t one) -> p t one", one=1)
                                .broadcast_to([P, NTg, P]),
                            iott[:].rearrange("p (one j) -> p one j", one=1)
                                .broadcast_to([P, NTg, P]),
                            op=mybir.AluOpType.is_equal)

                    if "elem" not in ABLATE:
                        # ee one-hot matmul chunks (4 tiles -> [128,512] PSUM)
                        CH = 4
                        for c0 in range(0, NTg, CH):
                            cn = min(CH, NTg - c0)
                            pee = pse.tile([P, CH * P], f32, tag="ee")
                            for k in range(cn):
                                nc.tensor.matmul(
                                    pee[:, k * P : (k + 1) * P],
                                    ohg[:, (c0 + k) * P : (c0 + k + 1) * P],
                                    eetb[:], start=True, stop=True,
                                    skip_group_check=True)
                            nc.vector.tensor_add(
                                g[:, c0 * P : (c0 + cn) * P],
                                g[:, c0 * P : (c0 + cn) * P],
                                pee[:, : cn * P])
                        # m = relu(z) into odd slots; e = exp(m) into even
                        # slots; me = m*e overwrites m slot
                        gme3 = gme[:].rearrange("p (t two j) -> p t two j",
                                                two=2, j=P)
                        mv = gme3[:, :NTg, 1, :]
                        ev = gme3[:, :NTg, 0, :]
                        gv = g[:, :EW].rearrange("p (t j) -> p t j", j=P)
                        nc.scalar.activation(mv, gv,
                                             mybir.ActivationFunctionType.Relu)
                        nc.scalar.activation(ev, mv,
                                             mybir.ActivationFunctionType.Exp)
                        nc.vector.tensor_mul(mv, mv, ev)

                    if "segmm" not in ABLATE:
                        bdone = 0
                        adh4 = ep.tile([P, TB * P], f32, tag="adh4")
                        for bi, b in enumerate(grp["blocks"]):
                            nlo_b, nhi_b = int(m.nlo_t[b]), int(m.nhi_t[b])
                            ntot = nlo_b + nhi_b
                            dw = m.dwin[b]
                            psEM = psp.tile([P, 2 * P], f32, tag="segEM")
                            wo = 0
                            for t in range(ntot):
                                lo, w = m.covers[b][t]
                                if t < nlo_b:
                                    gt = grp["lo_toff"][b] + t
                                else:
                                    gt = TLO + grp["hi_toff"][b] + (t - nlo_b)
                                nc.tensor.matmul(
                                    psEM[lo : lo + w, :],
                                    ind[:, gt * P : gt * P + w],
                                    gme[:, gt * 2 * P : (gt + 1) * 2 * P],
                                    start=(t == 0), stop=(t == ntot - 1),
                                    skip_group_check=True)

                            den = ep.tile([P, P], f32, tag="den")
                            nc.vector.tensor_scalar(
                                den[:dw, :], psEM[:dw, :P], betac[0:dw, :],
                                DEN_TINY, op0=mybir.AluOpType.mult,
                                op1=mybir.AluOpType.add)
                            nc.vector.reciprocal(den[:dw, :], den[:dw, :])
                            nc.vector.tensor_mul(adh4[:dw, bdone * P :
                                                      (bdone + 1) * P],
                                                 psEM[:dw, P : 2 * P],
                                                 den[:dw, :])
                            bdone += 1
                            if bdone == TB or bi == len(grp["blocks"]) - 1:
                                b0 = b - bdone + 1
                                pst = psp.tile([P, TB * P], f32, tag="tr")
                                for k in range(bdone):
                                    dwk = m.dwin[b0 + k]
                                    nc.tensor.matmul(
                                        pst[:, k * P : k * P + dwk],
                                        adh4[:dwk, k * P : (k + 1) * P],
                                        ident[:dwk, :dwk],
                                        start=True, stop=True,
                                        skip_group_check=True)
                                cw = (bdone - 1) * P + m.dwin[b0 + bdone - 1]
                                nc.scalar.activation(
                                    agg[:, b0 * P : b0 * P + cw],
                                    pst[:, :cw],
                                    mybir.ActivationFunctionType.Copy)
                                bdone = 0
                                adh4 = ep.tile([P, TB * P], f32, tag="adh4")

                if DEBUG and l == 0:
                    nc.sync.dma_start(dbg["dbg_hv1"].ap(), hv1[:])
                    nc.sync.dma_start(dbg["dbg_agg"].ap(), agg[:])

                # X = hv1 + agg  (into agg)
                nc.vector.tensor_add(agg[:], agg[:], hv1[:])

                # MLP + bias + skip
                mw = wp.tile([H, H], f32, tag="mw")
                nc.sync.dma_start(mw[:], inp["mlpw"].ap()[l * H:(l + 1) * H, :])
                mb = wp.tile([1, H], f32, tag="mb")
                nc.sync.dma_start(mb[:], inp["mlpb"].ap()[l : l + 1, :])
                for j in range(nch):
                    w = min(512, NPC - j * 512)
                    ps = psp.tile([P, 512], f32, tag="mm")
                    nc.tensor.matmul(ps[:, :w], mb[:], ones_row[:, :w],
                                     start=True, stop=False)
                    nc.tensor.matmul(ps[:, :w], mw[:],
                                     agg[:, j * 512 : j * 512 + w],
                                     start=False, stop=True)
                    nc.vector.tensor_add(hv[:, j * 512 : j * 512 + w],
                                         hv[:, j * 512 : j * 512 + w],
                                         ps[:, :w])

            if DEBUG:
                nc.sync.dma_start(dbg["dbg_hvl0"].ap(), hv[:])

            # ---------------- pooling + output ----------------
            psg = psp.tile([NGRAPH, H], f32, tag="segEM")
            psc = pse.tile([NGRAPH, 1], f32, tag="ee")
            for b in range(nblk):
                w = min(P, NPC - b * P)
                pst = psp.tile([P, TB * P], f32, tag="tr")
                nc.tensor.matmul(pst[:w, :P], hv[:, b * P : b * P + w],
                                 ident[:], start=True, stop=True)
                xs = wp.tile([P, P], f32, tag="xs")
                nc.scalar.activation(xs[:w, :], pst[:w, :P],
                                     mybir.ActivationFunctionType.Copy)
                pm = wp.tile([P, NGRAPH], f32, tag="pm")
                nc.sync.dma_start(pm[:], inp["pmat"].ap()[b * P:(b + 1) * P, :])
                nc.tensor.matmul(psg[:], pm[:w, :], xs[:w, :],
                                 start=(b == 0), stop=(b == nblk - 1))
                nc.tensor.matmul(psc[:], pm[:w, :], onecol[:w, :],
                                 start=(b == 0), stop=(b == nblk - 1))
            hgc = sp.tile([NGRAPH, H + 1], f32, tag="hgc")
            nc.scalar.activation(hgc[:, :H], psg[:],
                                 mybir.ActivationFunctionType.Copy)
            nc.scalar.activation(hgc[:, H : H + 1], psc[:],
                                 mybir.ActivationFunctionType.Copy)
            nc.sync.dma_start(hgin.ap(), hgc[:])
            if "coll" not in ABLATE:
                nc.gpsimd.collective_compute(
                    "AllReduce", mybir.AluOpType.add, replica_groups=RG,
                    ins=[hgin.ap()], outs=[hgout.ap()])
            hgg = sp.tile([NGRAPH, H + 1], f32, tag="hgg")
            nc.sync.dma_start(hgg[:], hgout.ap() if "coll" not in ABLATE
                              else hgin.ap())
            rcnt = sp.tile([NGRAPH, 1], f32, tag="rcnt")
            nc.vector.reciprocal(rcnt[:], hgg[:, H : H + 1])
            hgs = sp.tile([NGRAPH, H], f32, tag="hgs")
            nc.vector.tensor_scalar(hgs[:], hgg[:, :H], rcnt[:], None,
                                    op0=mybir.AluOpType.mult)
            pst = psp.tile([P, TB * P], f32, tag="tr")
            nc.tensor.matmul(pst[:, :NGRAPH], hgs[:], ident[:NGRAPH, :NGRAPH],
                             start=True, stop=True)
            hgT = sp.tile([P, NGRAPH], f32, tag="hgT")
            nc.scalar.activation(hgT[:], pst[:, :NGRAPH],
                                 mybir.ActivationFunctionType.Copy)
            pso = psp.tile([NGRAPH, OUT_DIM], f32, tag="mm")
            nc.tensor.matmul(pso[:], hgT[:], wout_sb[:], start=True, stop=True)
            ot = sp.tile([NGRAPH, OUT_DIM], f32, tag="ot")
            nc.vector.tensor_add(ot[:], bout_sb[:], pso[:])
            nc.sync.dma_start(out_t.ap(), ot[:])

    nc.compile()
    return nc


def run(inputs, trace=False):
    m, per_core = _preprocess(inputs)
    nc = _build(m)
    res = bass_utils.run_bass_kernel_spmd(
        nc, per_core, core_ids=list(range(N_CORES)), trace=trace)
    return np.asarray(res.results[0]["out"], np.float32), res


def kernel(**inputs) -> np.ndarray:
    out, _ = run(inputs)
    return out


if __name__ == "__main__":
    import reference as R
    inputs = {k: np.asarray(v) for k, v in R.setup_inputs().items()}
    out = kernel(**inputs)
    print(out.shape, out.dtype)


# revision 27
# speedup vs baseline: 1.2339x; 1.0764x over previous
"""Trainium2 Bass kernel for a 3-layer GENConv-style GNN (DGCN) on 8 NeuronCores.

Strategy (graph-partition data parallel):
  - Nodes are split contiguously across 8 cores (6250 nodes/core); each core owns
    all edges whose *destination* lies in its node range.
  - Node state hv lives in SBUF as [H=128 partitions, nodes] per core.
  - Per layer: tiny AllReduce of BatchNorm statistics -> BN+ReLU in one ScalarE
    activation pass -> transpose hv1 tiles (scaled by beta via a beta*I identity)
    into a DRAM shard -> AllGather into a per-core full gather table [N, H] ->
    edge pass: indirect-DMA gather of hv1[src] rows (two gathers over the
    int16-index table halves, rotated across 4 SWDGE queues), edge-embedding
    rows added via TensorE matmuls against a host-packed 18-row one-hot stream,
    ReLU and exp on ScalarE into an interleaved [e|me] layout, and segment sums
    via a single TensorE matmul per edge tile against 0/1 indicator tiles that
    are generated on-chip (broadcasted is_equal against an iota row) ->
    agg = (sum m*e) / (beta * sum e) -> MLP matmul (+bias via a rank-1 matmul)
    + skip.
  - Softmax max-subtraction is skipped (z = beta*m is bounded, exp can't
    overflow; result is shift-invariant); empty segments guarded with +1e-30.
  - Final average pooling via a host-built 0/1 pooling matrix on TensorE; the
    per-core partial [16, H+1] (sums + counts) is AllReduced, and every core
    computes the output Linear.

The program is SPMD (one instruction stream for all 8 cores), so all
data-dependent tiling metadata (tiles per dst-block, indicator widths) is
uniformized across cores by taking per-(block,tile) maxima; the actual indices
and indicator contents are per-core input data.
"""

import sys

sys.path.insert(0, "/opt/trn_rl_repo")

import numpy as np
import ml_dtypes

import concourse.bass as bass
import concourse.bacc as bacc
import concourse.tile as tile
import concourse.mybir as mybir
from concourse import bass_utils
from concourse.masks import make_identity

f32 = mybir.dt.float32
bf16 = mybir.dt.bfloat16
i32 = mybir.dt.int32

N_CORES = 8
H = 128
P = 128
OUT_DIM = 14
NGRAPH = 16
LAYERS = 3
BN_EPS = 1e-5
GEN_EPS = 1e-7
DEN_TINY = 1e-30
DEBUG = False
ABLATE = set()  # timing ablations: "gather","elem","segmm","coll"
NUM_QUEUES = 4       # SWDGE queues (1-4)
GT = 24              # tiles per gather group
TB = 4               # blocks per batched transpose
IND_BF16 = True      # generate indicators from bf16 iota/dloc (2x DVE)
ACT_RECIP = False    # den+recip on ScalarE (Reciprocal act is banned upstream)
LANE_SORT = True     # sort lanes within each tile by src (gather locality)
SCRATCH = 32768      # dynamic DMA scratch bytes/partition (SWDGE ring size)


class Meta:
    pass


def _preprocess(inputs):
    """Host-side index preprocessing + per-core input shards."""
    m = Meta()
    src = np.asarray(inputs["edge_src"], np.int64)
    dst = np.asarray(inputs["edge_dst"], np.int64)
    combo = (np.asarray(inputs["edge_feat0"], np.int64) * 3
             + np.asarray(inputs["edge_feat1"], np.int64))
    nf0 = np.asarray(inputs["node_feat0"], np.int64)
    nf1 = np.asarray(inputs["node_feat1"], np.int64)
    gids = np.asarray(inputs["graph_ids"], np.int64)

    N = nf0.shape[0]
    V0 = np.asarray(inputs["W_node0"]).shape[0]
    V1 = np.asarray(inputs["W_node1"]).shape[0]
    NPC = N // N_CORES
    nblk = (NPC + P - 1) // P
    m.N, m.NPC, m.nblk = N, NPC, nblk
    m.V0, m.V1 = V0, V1

    order = np.argsort(dst, kind="stable")
    dsts, srcs, combos = dst[order], src[order], combo[order]

    SPLIT = 32768  # int16 gather-index limit; table accessed as two halves
    rng = {}
    for c in range(N_CORES):
        for b in range(nblk):
            lo = c * NPC + b * P
            hi = min(c * NPC + min(NPC, (b + 1) * P), (c + 1) * NPC)
            i0 = np.searchsorted(dsts, lo, "left")
            i1 = np.searchsorted(dsts, hi, "left")
            rng[(c, b)] = (i0, i1)
    m.dwin = [min(P, NPC - b * P) for b in range(nblk)]

    # per-(core, block) edges reordered: (src < SPLIT, by dst), then (src >= SPLIT, by dst)
    # pad each half to a cross-core-uniform number of 128-edge tiles.
    core_blk = {}
    nlo_t = np.ones(nblk, dtype=np.int64)
    nhi_t = np.ones(nblk, dtype=np.int64)
    for c in range(N_CORES):
        for b in range(nblk):
            i0, i1 = rng[(c, b)]
            s_b = srcs[i0:i1]
            d_b = dsts[i0:i1] - (c * NPC + b * P)
            c_b = combos[i0:i1]
            is_hi = s_b >= SPLIT
            lo_sel = np.argsort(d_b[~is_hi], kind="stable")
            hi_sel = np.argsort(d_b[is_hi], kind="stable")
            parts = ((s_b[~is_hi][lo_sel], d_b[~is_hi][lo_sel], c_b[~is_hi][lo_sel]),
                     (s_b[is_hi][hi_sel] - SPLIT, d_b[is_hi][hi_sel], c_b[is_hi][hi_sel]))
            core_blk[(c, b)] = parts
            nlo_t[b] = max(nlo_t[b], (len(parts[0][0]) + P - 1) // P)
            nhi_t[b] = max(nhi_t[b], (len(parts[1][0]) + P - 1) // P)
    m.nlo_t, m.nhi_t = nlo_t, nhi_t
    NT = nlo_t + nhi_t
    m.NT = NT

    # uniform covers per (b, run, t): cross-core dst spans; block-first/last
    # tiles full-width (PSUM zero/close discipline); middles quadrant-aligned.
    covers = []
    for b in range(nblk):
        spans = []
        for run, ntr in ((0, int(nlo_t[b])), (1, int(nhi_t[b]))):
            for t in range(ntr):
                lo_u, hi_u = 1 << 30, 0
                for c in range(N_CORES):
                    dl = core_blk[(c, b)][run][1]
                    ch = dl[t * P:(t + 1) * P]
                    if len(ch):
                        lo_u = min(lo_u, int(ch.min()))
                        hi_u = max(hi_u, int(ch.max() + 1))
                spans.append((lo_u, hi_u))
        cv = []
        ntot = int(NT[b])
        for t in range(ntot):
            lo_u, hi_u = spans[t]
            if t == 0 or t == ntot - 1:
                cv.append((0, m.dwin[b]))
            else:
                lo = min(lo_u, m.dwin[b] - 1)
                hi = max(min(hi_u, m.dwin[b]), lo + 1)
                a32 = (lo // 32) * 32
                a64 = (lo // 64) * 64
                if hi <= a32 + 32 and a32 in (0, 32, 64):
                    cv.append((a32, 32))
                elif hi <= a64 + 64:
                    cv.append((a64, 64))
                else:
                    cv.append((0, P))
        covers.append(cv)
    m.covers = covers
    m.T = int(NT.sum())

    # group consecutive blocks for batched gathers
    groups, cur = [], None
    for b in range(nblk):
        if cur is None or cur["nt"] + int(NT[b]) > GT:
            cur = dict(blocks=[], nt=0, tlo=0, thi=0)
            groups.append(cur)
        cur["blocks"].append(b)
        cur["nt"] += int(NT[b])
        cur["tlo"] += int(nlo_t[b])
        cur["thi"] += int(nhi_t[b])
    for grp in groups:
        lt = ht = 0
        grp["lo_toff"], grp["hi_toff"] = {}, {}
        for b in grp["blocks"]:
            grp["lo_toff"][b] = lt; lt += int(nlo_t[b])
            grp["hi_toff"][b] = ht; ht += int(nhi_t[b])
    m.groups = groups
    # flat offsets: ixoff (idx stream, 16-wrapped cols), txoff (tile cols)
    ixoff, txoff = [], []
    ax = tx = 0
    for grp in groups:
        ixoff.append(ax)
        txoff.append(tx)
        ax += (grp["tlo"] + grp["thi"]) * 8
        tx += grp["tlo"] + grp["thi"]
    m.ixoff, m.IXW = ixoff, ax
    m.txoff = txoff
    assert tx == m.T

    def wrap16(seq):
        sq = seq.astype(np.int16).reshape(-1, 16).T        # [16, S]
        return np.tile(sq, (8, 1))                          # [128, S]

    # --- per-core data arrays ---
    per_core = []
    for c in range(N_CORES):
        gidx = np.zeros((P, m.IXW), np.int16)
        dloc = np.full((P, m.T), -1.0,
                       ml_dtypes.bfloat16 if IND_BF16 else np.float32)
        ohc = np.zeros((18, m.T * P), ml_dtypes.bfloat16)
        blk_streams = {}
        for b in range(nblk):
            (ls, ld, lc), (hs, hd, hc) = core_blk[(c, b)]
            streams = []
            for (s_, d_, c_), ntr in (((ls, ld, lc), int(nlo_t[b])),
                                      ((hs, hd, hc), int(nhi_t[b]))):
                npad = ntr * P - len(s_)
                sp_ = np.concatenate([s_, np.zeros(npad, np.int64)])
                dp_ = np.concatenate([d_, np.full(npad, -1, np.int64)])
                cp_ = np.concatenate([c_, np.zeros(npad, np.int64)])
                if LANE_SORT:
                    st = sp_.reshape(ntr, P)
                    perm = np.argsort(st, axis=1, kind="stable")
                    sp_ = np.take_along_axis(st, perm, 1).ravel()
                    dp_ = np.take_along_axis(dp_.reshape(ntr, P), perm, 1).ravel()
                    cp_ = np.take_along_axis(cp_.reshape(ntr, P), perm, 1).ravel()
                streams.append((sp_, dp_, cp_))
            blk_streams[b] = streams
        # idx streams per group + dloc + combo one-hot (all in g-tile order)
        for gi, grp in enumerate(groups):
            lo_seq = np.concatenate(
                [blk_streams[b][0][0] for b in grp["blocks"]])
            hi_seq = np.concatenate(
                [blk_streams[b][1][0] for b in grp["blocks"]])
            o = m.ixoff[gi]
            gidx[:, o : o + grp["tlo"] * 8] = wrap16(lo_seq)
            gidx[:, o + grp["tlo"] * 8 : o + (grp["tlo"] + grp["thi"]) * 8] = \
                wrap16(hi_seq)
            d_seq = np.concatenate(
                [blk_streams[b][0][1] for b in grp["blocks"]]
                + [blk_streams[b][1][1] for b in grp["blocks"]])
            c_seq = np.concatenate(
                [blk_streams[b][0][2] for b in grp["blocks"]]
                + [blk_streams[b][1][2] for b in grp["blocks"]])
            # per-tile cover lo offsets, in the same tile order
            lo_t = []
            for b in grp["blocks"]:
                lo_t.extend(cv[0] for cv in covers[b][: int(nlo_t[b])])
            for b in grp["blocks"]:
                lo_t.extend(
                    cv[0] for cv in covers[b][int(nlo_t[b]) : int(NT[b])])
            to = m.txoff[gi]
            ntg = grp["tlo"] + grp["thi"]
            dv = d_seq.reshape(ntg, P)
            lt = np.asarray(lo_t, np.int64).reshape(ntg, 1)
            dl = np.where(dv >= 0, dv - lt, -1).astype(dloc.dtype)
            dloc[:, to : to + ntg] = dl.T
            ohc[c_seq, np.arange(to * P, (to + ntg) * P)] = 1.0

        nlo, nhi = c * NPC, (c + 1) * NPC
        oh0 = np.zeros((V0, NPC), np.float32)
        oh0[nf0[nlo:nhi], np.arange(NPC)] = 1.0
        oh1 = np.zeros((V1, NPC), np.float32)
        oh1[nf1[nlo:nhi], np.arange(NPC)] = 1.0
        pmat = np.zeros((nblk * P, NGRAPH), np.float32)
        g_c = gids[nlo:nhi]
        for b in range(nblk):
            w = min(P, NPC - b * P)
            pmat[b * P : b * P + w, :][np.arange(w), g_c[b * P : b * P + w]] = 1.0
        per_core.append(dict(
            gidx=gidx, dloc=dloc, ohc=ohc,
            oh0=oh0, oh1=oh1, pmat=pmat,
        ))

    # replicated parameter tensors
    ee0 = np.asarray(inputs["edge_emb0"], np.float32)   # [L, 6, H]
    ee1 = np.asarray(inputs["edge_emb1"], np.float32)   # [L, 3, H]
    e0x = np.repeat(ee0, 3, axis=1)                      # [L, 18, H]
    e1x = np.tile(ee1, (1, 6, 1))                        # [L, 18, H]
    beta = np.asarray(inputs["beta"], np.float32)        # [L]
    colv = np.zeros((P, 3 * LAYERS), np.float32)
    for l in range(LAYERS):
        colv[:, 3 * l + 0] = np.asarray(inputs["bn_gamma"])[l]
        colv[:, 3 * l + 1] = np.asarray(inputs["bn_beta"])[l]
        colv[:, 3 * l + 2] = beta[l]
    mlpb_rows = np.asarray(inputs["mlp_b"], np.float32)      # [L, H]
    bout_rep = np.tile(np.asarray(inputs["b_out"], np.float32)[None, :],
                       (NGRAPH, 1))                          # [16, 14]
    shared = dict(
        wn0=np.asarray(inputs["W_node0"], np.float32),
        wn1=np.asarray(inputs["W_node1"], np.float32),
        e0x=e0x.reshape(LAYERS * 18, H), e1x=e1x.reshape(LAYERS * 18, H),
        colv=colv,
        mlpw=np.asarray(inputs["mlp_W"], np.float32).reshape(LAYERS * H, H),
        mlpb=mlpb_rows,
        wout=np.asarray(inputs["W_out"], np.float32),
        bout=bout_rep,
    )
    for d in per_core:
        d.update(shared)
    return m, per_core


def _build(m):
    NPC, nblk = m.NPC, m.nblk
    nc = bacc.Bacc("TRN2", target_bir_lowering=False, debug=False,
                   num_devices=N_CORES, num_swdge_queues=NUM_QUEUES,
                   dynamic_dma_scratch_size=SCRATCH)

    inp = {}
    def di(name, shape, dtype=f32):
        inp[name] = nc.dram_tensor(name, list(shape), dtype, kind="ExternalInput")
        return inp[name]

    di("gidx", [P, m.IXW], mybir.dt.int16)
    di("dloc", [P, m.T], bf16 if IND_BF16 else f32)
    di("ohc", [18, m.T * P], bf16)
    di("oh0", [m.V0, NPC]); di("oh1", [m.V1, NPC])
    di("pmat", [nblk * P, NGRAPH])
    di("wn0", [m.V0, H]); di("wn1", [m.V1, H])
    di("e0x", [LAYERS * 18, H]); di("e1x", [LAYERS * 18, H])
    di("colv", [P, 3 * LAYERS])
    di("mlpw", [LAYERS * H, H]); di("mlpb", [LAYERS, H])
    di("wout", [H, OUT_DIM]); di("bout", [NGRAPH, OUT_DIM])
    out_t = nc.dram_tensor("out", [NGRAPH, OUT_DIM], f32, kind="ExternalOutput")
    dbg = {}
    if DEBUG:
        for nm in ("dbg_hv0", "dbg_hv1", "dbg_agg", "dbg_hvl0"):
            dbg[nm] = nc.dram_tensor(nm, [P, NPC], f32, kind="ExternalOutput")

    table = nc.dram_tensor("table", [m.N, H], bf16, kind="Internal",
                           addr_space="Shared")
    aginb = nc.dram_tensor("aginb", [NPC, H], bf16, kind="Internal")
    arin = nc.dram_tensor("arin", [P, 2], f32, kind="Internal")
    arout = nc.dram_tensor("arout", [P, 2], f32, kind="Internal", addr_space="Shared")
    hgin = nc.dram_tensor("hgin", [NGRAPH, H + 1], f32, kind="Internal")
    hgout = nc.dram_tensor("hgout", [NGRAPH, H + 1], f32, kind="Internal",
                           addr_space="Shared")
    RG = [list(range(N_CORES))]

    NPC_pad = nblk * P

    with tile.TileContext(nc) as tc:
        with tc.tile_pool(name="persist", bufs=1) as pp, \
             tc.tile_pool(name="work", bufs=3) as wp, \
             tc.tile_pool(name="edge", bufs=2) as ep, \
             tc.tile_pool(name="small", bufs=2) as sp, \
             tc.tile_pool(name="ps", bufs=2, space="PSUM") as psp, \
             tc.tile_pool(name="psee", bufs=2, space="PSUM") as pse:

            # ---------------- constants ----------------
            ident = pp.tile([P, P], f32, tag="ident")
            make_identity(nc, ident[:])
            iott = pp.tile([P, P], bf16 if IND_BF16 else f32, tag="iott")
            nc.gpsimd.iota(iott[:], pattern=[[1, P]], base=0,
                           channel_multiplier=0,
                           allow_small_or_imprecise_dtypes=True)
            colv_sb = pp.tile([P, 3 * LAYERS], f32, tag="colv")
            nc.sync.dma_start(colv_sb[:], inp["colv"].ap())
            wn0_sb = pp.tile([m.V0, H], f32, tag="wn0")
            nc.sync.dma_start(wn0_sb[:], inp["wn0"].ap())
            wn1_sb = pp.tile([m.V1, H], f32, tag="wn1")
            nc.sync.dma_start(wn1_sb[:], inp["wn1"].ap())
            wout_sb = pp.tile([H, OUT_DIM], f32, tag="wout")
            nc.sync.dma_start(wout_sb[:], inp["wout"].ap())
            bout_sb = pp.tile([NGRAPH, OUT_DIM], f32, tag="bout")
            nc.sync.dma_start(bout_sb[:], inp["bout"].ap())
            ones_row = pp.tile([1, 512], f32, tag="ones")
            nc.vector.memset(ones_row[:], 1.0)
            onecol = pp.tile([P, 1], f32, tag="onecol")
            nc.vector.memset(onecol[:], 1.0)

            # persistent streams
            gidx_sb = pp.tile([P, m.IXW], mybir.dt.int16, tag="gidx")
            nc.sync.dma_start(gidx_sb[:], inp["gidx"].ap())
            dloc_sb = pp.tile([P, m.T], bf16 if IND_BF16 else f32, tag="dloc")
            nc.sync.dma_start(dloc_sb[:], inp["dloc"].ap())

            # persistent node state [H, NPC]
            hv = pp.tile([P, NPC], f32, tag="hv")
            hv1 = pp.tile([P, NPC], f32, tag="hv1")
            agg = pp.tile([P, NPC], f32, tag="agg")
            stage = pp.tile([P, NPC_pad], bf16, tag="stage")

            # ---------------- embedding ----------------
            nch = (NPC + 511) // 512
            for j in range(nch):
                w = min(512, NPC - j * 512)
                o0 = wp.tile([m.V0, 512], f32, tag="oh0")
                nc.sync.dma_start(o0[:, :w], inp["oh0"].ap()[:, j * 512 : j * 512 + w])
                o1 = wp.tile([m.V1, 512], f32, tag="oh1")
                nc.sync.dma_start(o1[:, :w], inp["oh1"].ap()[:, j * 512 : j * 512 + w])
                ps = psp.tile([P, 512], f32, tag="mm")
                nc.tensor.matmul(ps[:, :w], wn0_sb[:], o0[:, :w],
                                 start=True, stop=False)
                nc.tensor.matmul(ps[:, :w], wn1_sb[:], o1[:, :w],
                                 start=False, stop=True)
                nc.scalar.activation(hv[:, j * 512 : j * 512 + w], ps[:, :w],
                                     mybir.ActivationFunctionType.Copy)

            if DEBUG:
                nc.sync.dma_start(dbg["dbg_hv0"].ap(), hv[:])

            # ---------------- layers ----------------
            for l in range(LAYERS):
                gcol = colv_sb[:, 3 * l + 0 : 3 * l + 1]
                bcol = colv_sb[:, 3 * l + 1 : 3 * l + 2]
                betac = colv_sb[:, 3 * l + 2 : 3 * l + 3]

                # BN stats (partial) + AllReduce
                S = sp.tile([P, 1], f32, tag="S")
                nc.vector.reduce_sum(S[:], hv[:], axis=mybir.AxisListType.X)
                Q = sp.tile([P, 1], f32, tag="Q")
                nc.scalar.activation(stage[:, :NPC], hv[:],
                                     mybir.ActivationFunctionType.Square,
                                     accum_out=Q[:])
                sq = sp.tile([P, 2], f32, tag="sq")
                nc.vector.tensor_copy(sq[:, 0:1], S[:])
                nc.vector.tensor_copy(sq[:, 1:2], Q[:])
                nc.sync.dma_start(arin.ap(), sq[:])
                if "coll" not in ABLATE:
                    nc.gpsimd.collective_compute(
                        "AllReduce", mybir.AluOpType.add, replica_groups=RG,
                        ins=[arin.ap()], outs=[arout.ap()])
                sqg = sp.tile([P, 2], f32, tag="sqg")
                nc.sync.dma_start(sqg[:], arout.ap() if "coll" not in ABLATE
                                  else arin.ap())

                mean = sp.tile([P, 1], f32, tag="mean")
                nc.vector.tensor_scalar_mul(mean[:], sqg[:, 0:1], 1.0 / m.N)
                var = sp.tile([P, 1], f32, tag="var")
                nc.vector.tensor_scalar_mul(var[:], sqg[:, 1:2], 1.0 / m.N)
                msq = sp.tile([P, 1], f32, tag="msq")
                nc.vector.tensor_mul(msq[:], mean[:], mean[:])
                nc.vector.tensor_tensor(var[:], var[:], msq[:],
                                        op=mybir.AluOpType.subtract)
                nc.vector.tensor_scalar_add(var[:], var[:], BN_EPS)
                rv = sp.tile([P, 1], f32, tag="rv")
                nc.vector.reciprocal(rv[:], var[:])
                rsq = sp.tile([P, 1], f32, tag="rsq")
                nc.scalar.activation(rsq[:], rv[:],
                                     mybir.ActivationFunctionType.Sqrt)
                sc = sp.tile([P, 1], f32, tag="sc")
                nc.vector.tensor_mul(sc[:], gcol, rsq[:])
                tsh = sp.tile([P, 1], f32, tag="tsh")
                nc.vector.tensor_mul(tsh[:], mean[:], sc[:])
                nc.vector.tensor_tensor(tsh[:], bcol, tsh[:],
                                        op=mybir.AluOpType.subtract)
                bI = sp.tile([P, P], f32, tag="bI")
                nc.vector.tensor_scalar(bI[:], ident[:], betac, None,
                                        op0=mybir.AluOpType.mult)

                # hv1 = relu(sc*hv + tsh)
                nc.scalar.activation(hv1[:], hv[:],
                                     mybir.ActivationFunctionType.Relu,
                                     bias=tsh[:], scale=sc[:])

                # transpose beta*hv1 into stage (batched TB blocks per PSUM
                # tile + single copy), then DMA to aginb, AllGather
                for b0 in range(0, nblk, TB):
                    bn = min(TB, nblk - b0)
                    pst = psp.tile([P, TB * P], f32, tag="tr")
                    for k in range(bn):
                        b = b0 + k
                        w = min(P, NPC - b * P)
                        nc.tensor.matmul(pst[:w, k * P : (k + 1) * P],
                                         hv1[:, b * P : b * P + w],
                                         bI[:], start=True, stop=True)
                    nc.scalar.activation(stage[:, b0 * P : (b0 + bn) * P],
                                         pst[:, : bn * P],
                                         mybir.ActivationFunctionType.Copy)
                nfull = NPC // P
                ag_ap = aginb.ap()[: nfull * P, :].rearrange(
                    "(b p) h -> p b h", p=P)
                nc.sync.dma_start(ag_ap, stage[:, : nfull * P])
                if NPC % P:
                    w = NPC % P
                    nc.sync.dma_start(aginb.ap()[nfull * P :, :],
                                      stage[:w, nfull * P : nfull * P + P])
                if "coll" not in ABLATE:
                    nc.gpsimd.collective_compute(
                        "AllGather", mybir.AluOpType.bypass, replica_groups=RG,
                        ins=[aginb.ap()], outs=[table.ap()])

                # edge-embedding table: beta * (e0 + e1) kept in SBUF (bf16)
                e0 = sp.tile([18, H], f32, tag="e0")
                nc.sync.dma_start(e0[:], inp["e0x"].ap()[l * 18:(l + 1) * 18, :])
                e1 = sp.tile([18, H], f32, tag="e1")
                nc.sync.dma_start(e1[:], inp["e1x"].ap()[l * 18:(l + 1) * 18, :])
                eet = sp.tile([18, H], f32, tag="eet")
                nc.vector.tensor_add(eet[:], e0[:], e1[:])
                eetb = sp.tile([18, H], bf16, tag="eetb")
                nc.vector.tensor_scalar(eetb[:], eet[:], betac[0:18, :], None,
                                        op0=mybir.AluOpType.mult)

                # ---------------- edge pass ----------------
                SPLIT = 32768
                hi_table = table.ap()[SPLIT:, :] if m.N > SPLIT else table.ap()
                for gi, grp in enumerate(m.groups):
                    if "edge" in ABLATE:
                        break
                    TLO, THI = grp["tlo"], grp["thi"]
                    NTg = TLO + THI
                    EW = NTg * P
                    o = m.ixoff[gi]
                    to = m.txoff[gi]

                    g = ep.tile([P, GT * P], bf16, tag="g")
                    gme = ep.tile([P, GT * 2 * P], bf16, tag="gme")
                    if "gather" not in ABLATE:
                        # split each run into ~ring-sized pieces spread over
                        # all SWDGE queues so transfers parallelize
                        pieces = []
                        for base, cnt, tab in ((0, TLO, table.ap()),
                                               (TLO, THI, hi_table)):
                            nsp = min(NUM_QUEUES // 2, cnt) or 1
                            q, r = divmod(cnt, nsp)
                            off = 0
                            for k in range(nsp):
                                n_k = q + (1 if k < r else 0)
                                if n_k:
                                    pieces.append((base + off, n_k, tab))
                                    off += n_k
                        for pi, (toff, n_k, tab) in enumerate(pieces):
                            nc.gpsimd.dma_gather(
                                out_ap=g[:, toff * P : (toff + n_k) * P]
                                    .rearrange("p (n h) -> p n h", n=n_k),
                                in_ap=tab,
                                idxs_ap=gidx_sb[:, o + toff * 8 :
                                                o + (toff + n_k) * 8],
                                num_idxs=n_k * P, num_idxs_reg=n_k * P,
                                elem_size=H, single_packet=False,
                                queue_num=(len(pieces) * gi + pi) % NUM_QUEUES)

                    # combo one-hot stream + indicator generation (on-chip)
                    ohg = ep.tile([18, GT * P], bf16, tag="ohg")
                    nc.sync.dma_start(ohg[:, :EW],
                                      inp["ohc"].ap()[:, to * P : to * P + EW])
                    ind = ep.tile([P, GT * P], bf16, tag="ind")
                    if "segmm" not in ABLATE:
                        nc.vector.tensor_tensor(
                            ind[:].rearrange("p (t j) -> p t j", t=GT)[:, :NTg, :],
                            dloc_sb[:, to : to + NTg]
                                .rearrange("p (t one) -> p t one", one=1)
                                .broadcast_to([P, NTg, P]),
                            iott[:].rearrange("p (one j) -> p one j", one=1)
                                .broadcast_to([P, NTg, P]),
                            op=mybir.AluOpType.is_equal)

                    if "elem" not in ABLATE:
                        # ee one-hot matmul chunks (4 tiles -> [128,512] PSUM)
                        CH = 4
                        for c0 in range(0, NTg, CH):
                            cn = min(CH, NTg - c0)
                            pee = pse.tile([P, CH * P], f32, tag="ee")
                            for k in range(cn):
                                nc.tensor.matmul(
                                    pee[:, k * P : (k + 1) * P],
                                    ohg[:, (c0 + k) * P : (c0 + k + 1) * P],
                                    eetb[:], start=True, stop=True,
                                    skip_group_check=True)
                            nc.vector.tensor_add(
                                g[:, c0 * P : (c0 + cn) * P],
                                g[:, c0 * P : (c0 + cn) * P],
                                pee[:, : cn * P])
                        # m = relu(z) into odd slots; e = exp(m) into even
                        # slots; me = m*e overwrites m slot
                        gme3 = gme[:].rearrange("p (t two j) -> p t two j",
                                                two=2, j=P)
                        mv = gme3[:, :NTg, 1, :]
                        ev = gme3[:, :NTg, 0, :]
                        gv = g[:, :EW].rearrange("p (t j) -> p t j", j=P)
                        nc.scalar.activation(mv, gv,
                                             mybir.ActivationFunctionType.Relu)
                        nc.scalar.activation(ev, mv,
                                             mybir.ActivationFunctionType.Exp)
                        nc.vector.tensor_mul(mv, mv, ev)

                    if "segmm" not in ABLATE:
                        bdone = 0
                        adh4 = None
                        for bi, b in enumerate(grp["blocks"]):
                            if bdone == 0:
                                adh4 = ep.tile([P, TB * P], f32, tag="adh4")
                            nlo_b, nhi_b = int(m.nlo_t[b]), int(m.nhi_t[b])
                            ntot = nlo_b + nhi_b
                            dw = m.dwin[b]
                            psEM = psp.tile([P, 2 * P], f32, tag="segEM")
                            wo = 0
                            for t in range(ntot):
                                lo, w = m.covers[b][t]
                                if t < nlo_b:
                                    gt = grp["lo_toff"][b] + t
                                else:
                                    gt = TLO + grp["hi_toff"][b] + (t - nlo_b)
                                nc.tensor.matmul(
                                    psEM[lo : lo + w, :],
                                    ind[:, gt * P : gt * P + w],
                                    gme[:, gt * 2 * P : (gt + 1) * 2 * P],
                                    start=(t == 0), stop=(t == ntot - 1),
                                    skip_group_check=True)

                            den = ep.tile([P, P], f32, tag="den")
                            if ACT_RECIP:
                                nc.scalar.activation(
                                    den[:dw, :], psEM[:dw, :P],
                                    mybir.ActivationFunctionType.Reciprocal,
                                    bias=DEN_TINY, scale=betac[0:dw, :])
                            else:
                                nc.vector.tensor_scalar(
                                    den[:dw, :], psEM[:dw, :P], betac[0:dw, :],
                                    DEN_TINY, op0=mybir.AluOpType.mult,
                                    op1=mybir.AluOpType.add)
                                nc.vector.reciprocal(den[:dw, :], den[:dw, :])
                            nc.vector.tensor_mul(adh4[:dw, bdone * P :
                                                      (bdone + 1) * P],
                                                 psEM[:dw, P : 2 * P],
                                                 den[:dw, :])
                            bdone += 1
                            if bdone == TB or bi == len(grp["blocks"]) - 1:
                                b0 = b - bdone + 1
                                pst = psp.tile([P, TB * P], f32, tag="tr")
                                for k in range(bdone):
                                    dwk = m.dwin[b0 + k]
                                    nc.tensor.matmul(
                                        pst[:, k * P : k * P + dwk],
                                        adh4[:dwk, k * P : (k + 1) * P],
                                        ident[:dwk, :dwk],
                                        start=True, stop=True,
                                        skip_group_check=True)
                                cw = (bdone - 1) * P + m.dwin[b0 + bdone - 1]
                                nc.scalar.activation(
                                    agg[:, b0 * P : b0 * P + cw],
                                    pst[:, :cw],
                                    mybir.ActivationFunctionType.Copy)
                                bdone = 0

                if DEBUG and l == 0:
                    nc.sync.dma_start(dbg["dbg_hv1"].ap(), hv1[:])
                    nc.sync.dma_start(dbg["dbg_agg"].ap(), agg[:])

                # X = hv1 + agg  (into agg)
                nc.vector.tensor_add(agg[:], agg[:], hv1[:])

                # MLP + bias + skip
                mw = wp.tile([H, H], f32, tag="mw")
                nc.sync.dma_start(mw[:], inp["mlpw"].ap()[l * H:(l + 1) * H, :])
                mb = wp.tile([1, H], f32, tag="mb")
                nc.sync.dma_start(mb[:], inp["mlpb"].ap()[l : l + 1, :])
                for j in range(nch):
                    w = min(512, NPC - j * 512)
                    ps = psp.tile([P, 512], f32, tag="mm")
                    nc.tensor.matmul(ps[:, :w], mb[:], ones_row[:, :w],
                                     start=True, stop=False)
                    nc.tensor.matmul(ps[:, :w], mw[:],
                                     agg[:, j * 512 : j * 512 + w],
                                     start=False, stop=True)
                    nc.vector.tensor_add(hv[:, j * 512 : j * 512 + w],
                                         hv[:, j * 512 : j * 512 + w],
                                         ps[:, :w])

            if DEBUG:
                nc.sync.dma_start(dbg["dbg_hvl0"].ap(), hv[:])

            # ---------------- pooling + output ----------------
            psg = psp.tile([NGRAPH, H], f32, tag="segEM")
            psc = pse.tile([NGRAPH, 1], f32, tag="ee")
            for b in range(nblk):
                w = min(P, NPC - b * P)
                pst = psp.tile([P, TB * P], f32, tag="tr")
                nc.tensor.matmul(pst[:w, :P], hv[:, b * P : b * P + w],
                                 ident[:], start=True, stop=True)
                xs = wp.tile([P, P], f32, tag="xs")
                nc.scalar.activation(xs[:w, :], pst[:w, :P],
                                     mybir.ActivationFunctionType.Copy)
                pm = wp.tile([P, NGRAPH], f32, tag="pm")
                nc.sync.dma_start(pm[:], inp["pmat"].ap()[b * P:(b + 1) * P, :])
                nc.tensor.matmul(psg[:], pm[:w, :], xs[:w, :],
                                 start=(b == 0), stop=(b == nblk - 1))
                nc.tensor.matmul(psc[:], pm[:w, :], onecol[:w, :],
                                 start=(b == 0), stop=(b == nblk - 1))
            hgc = sp.tile([NGRAPH, H + 1], f32, tag="hgc")
            nc.scalar.activation(hgc[:, :H], psg[:],
                                 mybir.ActivationFunctionType.Copy)
            nc.scalar.activation(hgc[:, H : H + 1], psc[:],
                                 mybir.ActivationFunctionType.Copy)
            nc.sync.dma_start(hgin.ap(), hgc[:])
            if "coll" not in ABLATE:
                nc.gpsimd.collective_compute(
                    "AllReduce", mybir.AluOpType.add, replica_groups=RG,
                    ins=[hgin.ap()], outs=[hgout.ap()])
            hgg = sp.tile([NGRAPH, H + 1], f32, tag="hgg")
            nc.sync.dma_start(hgg[:], hgout.ap() if "coll" not in ABLATE
                              else hgin.ap())
            rcnt = sp.tile([NGRAPH, 1], f32, tag="rcnt")
            nc.vector.reciprocal(rcnt[:], hgg[:, H : H + 1])
            hgs = sp.tile([NGRAPH, H], f32, tag="hgs")
            nc.vector.tensor_scalar(hgs[:], hgg[:, :H], rcnt[:], None,
                                    op0=mybir.AluOpType.mult)
            pst = psp.tile([P, TB * P], f32, tag="tr")
            nc.tensor.matmul(pst[:, :NGRAPH], hgs[:], ident[:NGRAPH, :NGRAPH],
                             start=True, stop=True)
            hgT = sp.tile([P, NGRAPH], f32, tag="hgT")
            nc.scalar.activation(hgT[:], pst[:, :NGRAPH],
                                 mybir.ActivationFunctionType.Copy)
            pso = psp.tile([NGRAPH, OUT_DIM], f32, tag="mm")
            nc.tensor.matmul(pso[:], hgT[:], wout_sb[:], start=True, stop=True)
            ot = sp.tile([NGRAPH, OUT_DIM], f32, tag="ot")
            nc.vector.tensor_add(ot[:], bout_sb[:], pso[:])
            nc.sync.dma_start(out_t.ap(), ot[:])

    nc.compile()
    return nc


def run(inputs, trace=False):
    m, per_core = _preprocess(inputs)
    nc = _build(m)
    res = bass_utils.run_bass_kernel_spmd(
        nc, per_core, core_ids=list(range(N_CORES)), trace=trace)
    return np.asarray(res.results[0]["out"], np.float32), res


def kernel(**inputs) -> np.ndarray:
    out, _ = run(inputs)
    return out


if __name__ == "__main__":
    import reference as R
    inputs = {k: np.asarray(v) for k, v in R.setup_inputs().items()}
    out = kernel(**inputs)
    print(out.shape, out.dtype)
